# revision 16
# baseline (speedup 1.0000x reference)
"""DiffusedFarthestAttention Trainium2 kernel (8-core SPMD Bass/Tile).

Decomposition (B=4 batches x 2 halves -> 8 cores; pair (2b, 2b+1) handles batch b):
  Phase 1: to_basis, N-split.  xspec_partial[K,C] = sum_n (evecs[n,:]*mass[n])^T x[n,:]
           over this core's 16384 rows; AllReduce over the pair.
  Middle (head-split, 4 heads per core, all program-identical; split encoded in DATA):
           spectral coefs, x_farT = spec1^T-matmul, GroupNorm (stats via ones-matmuls),
           q/k/v projections in transposed layouts, scoresT per head, exp (unnormalized),
           PV with ones-augmented V giving denominators, normalization via DRAM-bounce
           broadcast, out-projection partial (bo/2 each) -> zspec_partial; AllReduce #2.
  Phase 3: from_basis, N-split.  out rows = evT_chunk^T @ (coefs_out*zspec*out_w).

All heavy matmuls run as float32r (FP22 truncation, 1 col/cycle with free dim >=256).
Host-side prep is layout-only (transposes, gathers by far_idx, reshapes); all
arithmetic (mass scaling, clamps, exp, norm, attention) happens on device.
"""

import numpy as np

import concourse.bass as bass
import concourse.mybir as mybir
import concourse.tile as tile
from concourse import bacc
from concourse.bass_utils import run_bass_kernel_spmd

B, N, K, M = 4, 32768, 128, 1024
C = 256          # C_IN = C_OUT = C_ATT
H, D = 8, 32     # heads, head dim
G = 32           # groupnorm groups (8 channels each)
EPS = 1e-6
P = 128
NH = N // 2      # rows per core
NCH = NH // P    # 128 n-chunks per core
P1G = 8          # n-chunks per P1 group
P3G = 8          # n-chunks per P3 group
HL = H // 2      # heads per core
F32 = mybir.dt.float32
F32R = mybir.dt.float32r
ADD = mybir.AluOpType.add
MULT = mybir.AluOpType.mult
AF = mybir.ActivationFunctionType


def _build():
    nc = bacc.Bacc("TRN2", target_bir_lowering=False, debug=False,
                   enable_asserts=False, num_devices=8)
    dt = F32
    T = lambda name, shape: nc.dram_tensor(name, list(shape), dt, kind="ExternalInput").ap()
    TR = lambda name, shape: nc.dram_tensor(name, list(shape), F32R, kind="ExternalInput").ap()
    x_h = TR("x_h", (NH, C))
    ev_h = TR("ev_h", (NH, K))
    evT_h = TR("evT_h", (K, NH))
    massT = T("massT", (P, NCH))
    evfar = TR("evfar", (M, K))
    mfarT = T("mfarT", (P, M // P))
    evTfar = TR("evTfar", (K, M))
    wq = TR("wq", (2, P, P))      # Wq[:, my 128 cols].reshape(2,128,128)
    wk = TR("wk", (2, P, P))
    wv = TR("wv", (2, P, C))      # Wv with my-half columns first
    wo = TR("wo", (HL, D, C))     # my 128 rows of Wo, split per head
    bq_c = T("bq_c", (P, 1))
    bk_c = T("bk_c", (P, 1))
    bv_r = T("bv_r", (1, P))     # my half of bv
    bo_r = T("bo_r", (1, C))     # bo * 0.5
    gnw = T("gnw", (P, 2))
    gnb = T("gnb", (P, 2))
    evals_c = T("evals_c", (P, 1))
    tin_r = T("tin_r", (1, C))
    tout_r = T("tout_r", (1, C))
    outw_r = T("outw_r", (1, C))
    gsum = TR("gsum", (P, 16))    # channel -> group indicator (per 128-chunk)
    konst = TR("konst", (2, HL * (D + 1)))   # row0 ones, row1 zeros
    maskq = T("maskq", (P, HL))   # head-row indicators
    bkm = T("bkm", (P, HL))       # bk * maskq
    gbp = TR("gbp", (P, P))       # padded group -> channel broadcast matrix
    out_ap = nc.dram_tensor("out", [NH, C], dt, kind="ExternalOutput").ap()

    RG = [[0, 1], [2, 3], [4, 5], [6, 7]]

    with tile.TileContext(nc) as tc:
        with tc.tile_pool(name="const", bufs=1) as cst, \
             tc.tile_pool(name="mid", bufs=3) as mid, \
             tc.tile_pool(name="dram", bufs=1, space="DRAM") as dram:

            # ---- persistent params / small tiles ----
            massT_t = cst.tile([P, NCH], dt, tag="massT")
            nc.sync.dma_start(massT_t[:], massT[:])
            evTfar_t = cst.tile([K, M], F32R, tag="evTfar")
            nc.sync.dma_start(evTfar_t[:], evTfar[:])
            wq_t = [cst.tile([P, P], F32R, tag=f"wq{j}", name=f"wq{j}") for j in range(2)]
            wk_t = [cst.tile([P, P], F32R, tag=f"wk{j}", name=f"wk{j}") for j in range(2)]
            wv_t = [cst.tile([P, C], F32R, tag=f"wv{j}", name=f"wv{j}") for j in range(2)]
            for j in range(2):
                nc.sync.dma_start(wq_t[j][:], wq[j])
                nc.sync.dma_start(wk_t[j][:], wk[j])
                nc.sync.dma_start(wv_t[j][:], wv[j])
            wo_t = [cst.tile([D, C], F32R, tag=f"wo{h}", name=f"wo{h}") for h in range(HL)]
            for h in range(HL):
                nc.sync.dma_start(wo_t[h][:], wo[h])
            bq_t = cst.tile([P, 1], dt, tag="bq")
            nc.sync.dma_start(bq_t[:], bq_c[:])
            bv_b = cst.tile([P, P], dt, tag="bvb")
            nc.sync.dma_start(bv_b[:], bv_r.to_broadcast([P, P]))
            bo_b = cst.tile([P, C], dt, tag="bob")
            nc.sync.dma_start(bo_b[:], bo_r.to_broadcast([P, C]))
            gnw_t = cst.tile([P, 2], dt, tag="gnw")
            nc.sync.dma_start(gnw_t[:], gnw[:])
            gnb_t = cst.tile([P, 2], dt, tag="gnb")
            nc.sync.dma_start(gnb_t[:], gnb[:])
            evals_t = cst.tile([P, 1], dt, tag="evals")
            nc.sync.dma_start(evals_t[:], evals_c[:])
            gsum_t = cst.tile([P, 16], F32R, tag="gsum")
            nc.sync.dma_start(gsum_t[:], gsum[:])
            gbp_t = cst.tile([P, P], F32R, tag="gbp")
            nc.sync.dma_start(gbp_t[:], gbp[:])
            mfarT_t = cst.tile([P, M // P], dt, tag="mfarT")
            nc.sync.dma_start(mfarT_t[:], mfarT[:])
            maskq_t = cst.tile([P, HL], dt, tag="maskq")
            nc.sync.dma_start(maskq_t[:], maskq[:])
            bkm_t = cst.tile([P, HL], dt, tag="bkm")
            nc.sync.dma_start(bkm_t[:], bkm[:])

            # clamped time vectors / out_w, broadcast over partitions
            tin_b = cst.tile([P, C], dt, tag="tinb")
            nc.sync.dma_start(tin_b[:], tin_r.to_broadcast([P, C]))
            nc.vector.tensor_scalar_max(tin_b[:], tin_b[:], 1e-8)
            tout_b = cst.tile([P, C], dt, tag="toutb")
            nc.sync.dma_start(tout_b[:], tout_r.to_broadcast([P, C]))
            nc.vector.tensor_scalar_max(tout_b[:], tout_b[:], 1e-8)
            outw_b = cst.tile([P, C], dt, tag="outwb")
            nc.sync.dma_start(outw_b[:], outw_r.to_broadcast([P, C]))
            nc.vector.tensor_scalar_max(outw_b[:], outw_b[:], 1e-8)

            # coefs = exp(-evals x t)
            coef_in = cst.tile([P, C], dt, tag="coefin")
            nc.vector.tensor_tensor(coef_in[:], evals_t[:].to_broadcast([P, C]),
                                    tin_b[:], MULT)
            nc.scalar.activation(coef_in[:], coef_in[:], AF.Exp, scale=-1.0)
            coef_out = cst.tile([P, C], dt, tag="coefout")
            nc.vector.tensor_tensor(coef_out[:], evals_t[:].to_broadcast([P, C]),
                                    tout_b[:], MULT)
            nc.scalar.activation(coef_out[:], coef_out[:], AF.Exp, scale=-1.0)

            # =============== PHASE 1: to_basis (N-split) ===============
            with tc.tile_pool(name="p1x", bufs=3) as p1x, \
                 tc.tile_pool(name="p1e", bufs=3) as p1e, \
                 tc.tile_pool(name="ps1", bufs=1, space="PSUM") as ps1:
                xspec_ps = ps1.tile([K, C], dt, tag="xspec")
                ng = NCH // P1G
                for g in range(ng):
                    xt = p1x.tile([P, P1G, C], F32R, tag="x8")
                    nc.sync.dma_start(
                        xt[:], x_h[g * P1G * P:(g + 1) * P1G * P, :]
                        .rearrange("(j p) c -> p j c", p=P))
                    et = p1e.tile([P, P1G, K], F32R, tag="e8")
                    nc.sync.dma_start(
                        et[:], ev_h[g * P1G * P:(g + 1) * P1G * P, :]
                        .rearrange("(j p) k -> p j k", p=P))
                    # scale evecs rows by mass (per-partition per chunk)
                    nc.vector.tensor_tensor(
                        et[:], et[:],
                        massT_t[:, g * P1G:(g + 1) * P1G, None].to_broadcast([P, P1G, K]),
                        MULT)
                    for j in range(P1G):
                        nc.tensor.matmul(xspec_ps[:], et[:, j, :], xt[:, j, :],
                                         start=(g == 0 and j == 0),
                                         stop=(g == ng - 1 and j == P1G - 1))
                xspec_sb = cst.tile([K, C], dt, tag="xspec_sb")
                nc.scalar.copy(xspec_sb[:], xspec_ps[:])

            # AllReduce #1 (pair): xspec
            ar1_in = dram.tile([K, C], dt, tag="ar1in")
            ar1_out = dram.tile([K, C], dt, tag="ar1out")
            nc.sync.dma_start(ar1_in[:], xspec_sb[:])
            nc.gpsimd.collective_compute(
                "AllReduce", ADD, replica_groups=RG,
                ins=[ar1_in[:].opt()], outs=[ar1_out[:].opt()])
            xspec_sum = cst.tile([K, C], dt, tag="xspec_sum")
            nc.sync.dma_start(xspec_sum[:], ar1_out[:])

            spec1 = cst.tile([K, C], F32R, tag="spec1")
            nc.vector.tensor_tensor(spec1[:], coef_in[:], xspec_sum[:], MULT)

            # =============== MIDDLE ===============
            with tc.tile_pool(name="psm", bufs=1, space="PSUM") as psm:
                # ---- x_farT [C(2 chunks of 128), M] + GN stats ----
                xfT = [cst.tile([P, M], F32R, tag=f"xfT{cc}", name=f"xfT{cc}") for cc in range(2)]
                rs_tmp = cst.tile([P, 8], dt, tag="rstmp")
                sq_scr = mid.tile([P, 512], dt, tag="sqscr")
                for cc in range(2):
                    for mh in range(2):
                        px = psm.tile([P, 512], dt, tag="mm512", bufs=2)
                        nc.tensor.matmul(px[:], spec1[:, cc * P:(cc + 1) * P],
                                         evTfar_t[:, mh * 512:(mh + 1) * 512],
                                         start=True, stop=True)
                        idx = cc * 2 + mh
                        nc.scalar.activation(xfT[cc][:, mh * 512:(mh + 1) * 512], px[:],
                                             AF.Copy, accum_out=rs_tmp[:, idx:idx + 1])
                        sq_scr = mid.tile([P, 512], dt, tag="sqscr")
                        nc.scalar.activation(sq_scr[:], px[:], AF.Square,
                                             accum_out=rs_tmp[:, 4 + idx:5 + idx])
                # group stats
                rsq = cst.tile([P, 4], F32R, tag="rsq")
                for cc in range(2):
                    nc.vector.tensor_add(rsq[:, cc:cc + 1], rs_tmp[:, 2 * cc:2 * cc + 1],
                                         rs_tmp[:, 2 * cc + 1:2 * cc + 2])
                    nc.vector.tensor_add(rsq[:, 2 + cc:3 + cc],
                                         rs_tmp[:, 4 + 2 * cc:5 + 2 * cc],
                                         rs_tmp[:, 5 + 2 * cc:6 + 2 * cc])
                pg = psm.tile([16, 4], dt, tag="tiny")
                nc.tensor.matmul(pg[:], gsum_t[:], rsq[:], start=True, stop=True)
                inv = 1.0 / (M * 8)
                mu = cst.tile([16, 2], dt, tag="mu")
                nc.vector.tensor_scalar_mul(mu[:], pg[:, 0:2], inv)
                ms = cst.tile([16, 2], dt, tag="ms")
                nc.vector.tensor_scalar_mul(ms[:], pg[:, 2:4], inv)
                var = cst.tile([16, 2], dt, tag="var")
                nc.vector.tensor_tensor(var[:], mu[:], mu[:], MULT)
                nc.vector.tensor_sub(var[:], ms[:], var[:])
                nc.vector.tensor_scalar_add(var[:], var[:], EPS)
                std = cst.tile([16, 2], dt, tag="std")
                nc.scalar.activation(std[:], var[:], AF.Sqrt)
                rstd = cst.tile([16, 2], dt, tag="rstd")
                nc.vector.reciprocal(rstd[:], std[:])
                stats_sb = cst.tile([P, 4], F32R, tag="stats")
                nc.sync.dma_start(stats_sb[:], konst[1:2, 0:4].to_broadcast([P, 4]))
                nc.vector.tensor_copy(out=stats_sb[0:16, 0:2], in_=mu[:])
                nc.vector.tensor_copy(out=stats_sb[0:16, 2:4], in_=rstd[:])
                pbc = psm.tile([P, 4], dt, tag="tiny")
                nc.tensor.matmul(pbc[:], gbp_t[:], stats_sb[:], start=True, stop=True)
                A = cst.tile([P, 2], dt, tag="gnA")
                nc.vector.tensor_tensor(A[:], pbc[:, 2:4], gnw_t[:], MULT)
                Bt = cst.tile([P, 2], dt, tag="gnB")
                nc.vector.tensor_tensor(Bt[:], pbc[:, 0:2], A[:], MULT)
                nc.vector.tensor_sub(Bt[:], gnb_t[:], Bt[:])
                for cc in range(2):
                    nc.vector.scalar_tensor_tensor(
                        xfT[cc][:], xfT[cc][:], A[:, cc:cc + 1],
                        Bt[:, cc:cc + 1].to_broadcast([P, M]), MULT, ADD)

                # ---- qT / kTpad projections ----
                qT = cst.tile([P, M], F32R, tag="qT")
                kTpad = [cst.tile([P, M], F32R, tag=f"kTpad{h}", name=f"kTpad{h}") for h in range(HL)]
                for mh in range(2):
                    pq = psm.tile([P, 512], dt, tag="mm512", bufs=2)
                    pk = psm.tile([P, 512], dt, tag="mm512", bufs=2)
                    for cin in range(2):
                        nc.tensor.matmul(pq[:], wq_t[cin][:],
                                         xfT[cin][:, mh * 512:(mh + 1) * 512],
                                         start=(cin == 0), stop=(cin == 1))
                    for cin in range(2):
                        nc.tensor.matmul(pk[:], wk_t[cin][:],
                                         xfT[cin][:, mh * 512:(mh + 1) * 512],
                                         start=(cin == 0), stop=(cin == 1))
                    nc.scalar.activation(qT[:, mh * 512:(mh + 1) * 512], pq[:],
                                         AF.Identity, bias=bq_t[:])
                    for h in range(HL):
                        # masked write: rows of head h get k+bk, others 0
                        nc.scalar.activation(kTpad[h][:, mh * 512:(mh + 1) * 512],
                                             pk[:, :], AF.Identity,
                                             bias=bkm_t[:, h:h + 1],
                                             scale=maskq_t[:, h:h + 1])

                # ---- v (natural, my-half cols first) + ones-augmented tiles ----
                # per-head 33-col blocks [v|1]; ones column gives softmax denom
                vaug = [cst.tile([P, HL * (D + 1)], F32R, tag=f"vaug{mc}", name=f"vaug{mc}")
                        for mc in range(M // P)]
                for mc in range(M // P):
                    nc.sync.dma_start(vaug[mc][:],
                                      konst[0:1, :].to_broadcast([P, HL * (D + 1)]))
                    pv = psm.tile([P, C], dt, tag="mm256", bufs=2)
                    for cin in range(2):
                        nc.tensor.matmul(pv[:], xfT[cin][:, mc * P:(mc + 1) * P],
                                         wv_t[cin][:],
                                         start=(cin == 0), stop=(cin == 1))
                    for h in range(HL):
                        nc.vector.tensor_add(vaug[mc][:, h * (D + 1):h * (D + 1) + D],
                                             pv[:, h * D:(h + 1) * D],
                                             bv_b[:, h * D:(h + 1) * D])

                # ---- attention: scoresT -> exp -> PV (unnormalized) ----
                # all per-head tiles live at partition base 0 (engine ops need
                # matching start partitions; psum matmul base must be 0/32/64)
                oTh = [cst.tile([D, M], F32R, tag=f"oTh{h}", name=f"oTh{h}")
                       for h in range(HL)]
                den_sb = cst.tile([D + 1, HL * M], dt, tag="densb")
                den_d = dram.tile([1, HL * M], dt, tag="dend")
                scl = 1.0 / np.sqrt(D)
                for h in range(HL):
                    for mqh in range(2):
                        po = psm.tile([D + 1, 512], dt, tag="po", bufs=2)
                        for mkc in range(M // P):
                            psc = psm.tile([P, 512], dt, tag="mm512", bufs=2)
                            nc.tensor.matmul(
                                psc[:], kTpad[h][:, mkc * P:(mkc + 1) * P],
                                qT[:, mqh * 512:(mqh + 1) * 512],
                                start=True, stop=True)
                            pt = mid.tile([P, 512], F32R, tag="ptile")
                            nc.scalar.activation(pt[:], psc[:], AF.Exp, scale=scl)
                            nc.tensor.matmul(
                                po[:], vaug[mkc][:, h * (D + 1):(h + 1) * (D + 1)],
                                pt[:],
                                start=(mkc == 0), stop=(mkc == M // P - 1))
                        sl = slice(mqh * 512, (mqh + 1) * 512)
                        nc.scalar.copy(oTh[h][:, sl], po[0:D, :])
                        dsl = slice((h * 2 + mqh) * 512, (h * 2 + mqh + 1) * 512)
                        nc.vector.tensor_copy(out=den_sb[D:D + 1, dsl],
                                              in_=po[D:D + 1, :])
                nc.sync.dma_start(den_d[:], den_sb[D:D + 1, :])
                # normalize oTh: broadcast 1/denom over d via DRAM bounce
                for h in range(HL):
                    db = cst.tile([D, M], dt, tag=f"denb{h}", name=f"denb{h}")
                    nc.sync.dma_start(
                        db[:], den_d[0:1, h * M:(h + 1) * M].to_broadcast([D, M]))
                    nc.vector.reciprocal(db[:], db[:])
                    nc.vector.tensor_tensor(oTh[h][:], oTh[h][:], db[:], MULT)

                # ---- out-projection partial + zspec partial ----
                zspec_ps = psm.tile([K, C], dt, tag="mm256z")
                for mc in range(M // P):
                    pa = psm.tile([P, C], dt, tag="mm256", bufs=2)
                    for h in range(HL):
                        nc.tensor.matmul(pa[:], oTh[h][:, mc * P:(mc + 1) * P],
                                         wo_t[h][:],
                                         start=(h == 0), stop=(h == HL - 1))
                    at = mid.tile([P, C], F32R, tag="atile")
                    nc.vector.tensor_add(at[:], pa[:], bo_b[:])
                    ef = mid.tile([P, K], F32R, tag="ef")
                    nc.sync.dma_start(ef[:], evfar[mc * P:(mc + 1) * P, :])
                    nc.vector.tensor_scalar_mul(ef[:], ef[:], mfarT_t[:, mc:mc + 1])
                    nc.tensor.matmul(zspec_ps[:], ef[:], at[:],
                                     start=(mc == 0), stop=(mc == M // P - 1))
                zspec_sb = cst.tile([K, C], dt, tag="zspec_sb")
                nc.scalar.copy(zspec_sb[:], zspec_ps[:])

            # AllReduce #2 (pair): zspec
            ar2_in = dram.tile([K, C], dt, tag="ar2in")
            ar2_out = dram.tile([K, C], dt, tag="ar2out")
            nc.sync.dma_start(ar2_in[:], zspec_sb[:])
            nc.gpsimd.collective_compute(
                "AllReduce", ADD, replica_groups=RG,
                ins=[ar2_in[:].opt()], outs=[ar2_out[:].opt()])
            zspec_sum = cst.tile([K, C], dt, tag="zspec_sum")
            nc.sync.dma_start(zspec_sum[:], ar2_out[:])

            spec2 = cst.tile([K, C], F32R, tag="spec2")
            nc.vector.tensor_tensor(spec2[:], coef_out[:], zspec_sum[:], MULT)
            nc.vector.tensor_tensor(spec2[:], spec2[:], outw_b[:], MULT)

            # =============== PHASE 3: from_basis (N-split) ===============
            with tc.tile_pool(name="p3e", bufs=3) as p3e, \
                 tc.tile_pool(name="p3o", bufs=3) as p3o, \
                 tc.tile_pool(name="ps3", bufs=6, space="PSUM") as ps3:
                ng = NCH // P3G
                for g in range(ng):
                    et = p3e.tile([K, P3G * P], F32R, tag="evt8")
                    nc.sync.dma_start(et[:], evT_h[:, g * P3G * P:(g + 1) * P3G * P])
                    ot = p3o.tile([P, P3G, C], dt, tag="out8")
                    for j in range(P3G):
                        pp = ps3.tile([P, C], dt, tag="p3")
                        nc.tensor.matmul(pp[:], et[:, j * P:(j + 1) * P],
                                         spec2[:], start=True, stop=True)
                        if j % 2 == 0:
                            nc.vector.tensor_copy(out=ot[:, j, :], in_=pp[:])
                        else:
                            nc.scalar.copy(ot[:, j, :], pp[:])
                    nc.sync.dma_start(
                        out_ap[g * P3G * P:(g + 1) * P3G * P, :]
                        .rearrange("(j p) c -> p j c", p=P),
                        ot[:])

    nc.compile()
    return nc


_PROG = None


def _get_prog():
    global _PROG
    if _PROG is None:
        _PROG = _build()
    return _PROG


def make_in_maps(x, mass, evals, evecs, far_idx, diff_in_t, diff_out_t, gn_w, gn_b,
                 Wq, bq, Wk, bk, Wv, bv, Wo, bo, out_w):
    """Host-side (layout-only) prep of the 8 per-core input dicts."""
    f32 = np.float32
    asf = lambda a: np.ascontiguousarray(a, dtype=f32)
    x = np.asarray(x, dtype=f32)
    mass = np.asarray(mass, dtype=f32)
    evals = np.asarray(evals, dtype=f32)
    evecs = np.asarray(evecs, dtype=f32)
    far_idx = np.asarray(far_idx)
    gsum_m = np.zeros((P, 16), f32)
    gsum_m[np.arange(P), np.arange(P) // 8] = 1.0
    gbp_m = np.zeros((P, P), f32)
    gbp_m[np.arange(P) // 8, np.arange(P)] = 1.0
    in_maps = []
    for core in range(8):
        b, half = core // 2, core % 2
        rs = slice(half * NH, (half + 1) * NH)
        hc = slice(half * P, (half + 1) * P)       # my C_ATT columns / heads
        oc = slice((1 - half) * P, (2 - half) * P)  # partner's columns
        fi = far_idx[b]
        ev_far = evecs[b][fi]                       # [M, K]
        m = {
            "x_h": asf(x[b, rs]),
            "ev_h": asf(evecs[b, rs]),
            "evT_h": asf(evecs[b, rs].T),
            "massT": asf(mass[b, rs].reshape(NCH, P).T),
            "evfar": asf(ev_far),
            "mfarT": asf(mass[b][fi].reshape(M // P, P).T),
            "evTfar": asf(ev_far.T),
            "wq": asf(Wq[:, hc].reshape(2, P, P)),
            "wk": asf(Wk[:, hc].reshape(2, P, P)),
            "wv": asf(np.concatenate([Wv[:, hc], Wv[:, oc]], axis=1).reshape(2, P, C)),
            "wo": asf(Wo[hc].reshape(HL, D, C)),
            "bq_c": asf(np.asarray(bq)[hc].reshape(P, 1)),
            "bk_c": asf(np.asarray(bk)[hc].reshape(P, 1)),
            "konst": np.stack([np.ones(HL * (D + 1), np.float32),
                               np.zeros(HL * (D + 1), np.float32)]),
            "maskq": asf((np.arange(P)[:, None] // D == np.arange(HL)[None, :])),
            "bkm": asf((np.arange(P)[:, None] // D == np.arange(HL)[None, :])
                       * np.asarray(bk)[hc][:, None]),
            "bv_r": asf(np.asarray(bv)[hc].reshape(1, P)),
            "bo_r": asf(0.5 * np.asarray(bo).reshape(1, C)),
            "gnw": asf(np.asarray(gn_w).reshape(2, P).T),
            "gnb": asf(np.asarray(gn_b).reshape(2, P).T),
            "evals_c": asf(evals[b].reshape(P, 1)),
            "tin_r": asf(np.asarray(diff_in_t).reshape(1, C)),
            "tout_r": asf(np.asarray(diff_out_t).reshape(1, C)),
            "outw_r": asf(np.asarray(out_w).reshape(1, C)),
            "gsum": gsum_m,
            "gbp": gbp_m,
        }
        in_maps.append(m)
    return in_maps


def kernel(**inputs):
    nc = _get_prog()
    in_maps = make_in_maps(**inputs)
    res = run_bass_kernel_spmd(nc, in_maps, core_ids=list(range(8)))
    out = np.empty((B, N, C), np.float32)
    for core in range(8):
        b, half = core // 2, core % 2
        out[b, half * NH:(half + 1) * NH] = res.results[core]["out"]
    return out


# revision 25
# speedup vs baseline: 17832.7568x; 17832.7568x over previous
"""DiffusedFarthestAttention Trainium2 kernel (8-core SPMD Bass/Tile).

Decomposition (B=4 batches x 2 halves -> 8 cores; pair (2b, 2b+1) handles batch b):
  Phase 1: to_basis, N-split.  xspec_partial[K,C] = sum_n (evecs[n,:]*mass[n])^T x[n,:]
           over this core's 16384 rows; AllReduce over the pair.  The evfar Gram
           matrix + column sums (for spectral GroupNorm stats) hide under P1's DMA.
  Middle (head-split, 4 heads per core; all 8 programs identical, split lives in
           the DATA): spectral coefs, GroupNorm stats computed spectrally from
           spec1 and the Gram matrix, x_farT via spec1-matmuls, q/k/v projections
           in transposed layouts, per-head scoresT -> single 1024-wide exp ->
           ones-augmented PV giving softmax denominators, denominator broadcast
           via small DRAM bounce, out-projection partial (bo/2 each) ->
           zspec_partial; AllReduce #2.
  Phase 3: from_basis, N-split.  out rows = evT_chunk^T @ (coefs_out*zspec*out_w).
           evT fully prefetched during the middle; per-partition-contiguous
           row-block layouts keep every DMA at >=2KB descriptors.

Heavy matmuls run as bfloat16 (P1/P3 streams) or float32r (FP22) elsewhere.
Host-side prep is layout-only (transposes, gathers by far_idx, reshapes, dtype
casts); all arithmetic happens on device.
"""

import numpy as np

import concourse.bass as bass
import concourse.mybir as mybir
import concourse.tile as tile
from concourse import bacc
from concourse.bass_utils import run_bass_kernel_spmd

B, N, K, M = 4, 32768, 128, 1024
C = 256          # C_IN = C_OUT = C_ATT
H, D = 8, 32     # heads, head dim
EPS = 1e-6
P = 128
NH = N // 2      # rows per core
NCH = NH // P    # 128 n-chunks per core
P1G = 8          # n-chunks per P1 group
P3G = 8          # n-chunks per P3 group
HL = H // 2      # heads per core
NMK = M // P
F32 = mybir.dt.float32
F32R = mybir.dt.float32r
BF16 = mybir.dt.bfloat16
DT1 = BF16       # phase-1 stream dtype (x, evecs natural)
DT3 = BF16       # phase-3 stream dtype (evT, spec2w)
P3E_BUFS = 16 if DT3 == BF16 else 10
ADD = mybir.AluOpType.add
MULT = mybir.AluOpType.mult
AF = mybir.ActivationFunctionType

# packed f32 param columns (pk1)
_PK1 = dict(massT=(0, NCH), mfarT=(128, NMK), maskq=(136, HL), bkm=(140, HL),
            gnw=(144, 2), gnb=(146, 2), evals=(148, 1), bq=(149, 1))
PK1_W = 150
# packed f32r matrix columns (pkr)
_PKR = dict(gsum=(0, 16), gbp=(16, P))
PKR_W = 144
# packed f32r weight columns (pkw): wq0|wq1|wk0|wk1|wv0|wv1
PKW_W = 2 * P + 2 * P + 2 * C


def _build(single=False, phases=(1, 2, 3), reps=1, noar=False):
    """single=True: 1-core variant with AllReduce -> local copy, for TimelineSim."""
    nc = bacc.Bacc("TRN2", target_bir_lowering=False, debug=False,
                   enable_asserts=False, num_devices=1 if single else 8)
    dt = F32
    x_h = nc.dram_tensor("x_h", [NH, C], DT1, kind="ExternalInput").ap()
    ev_h = nc.dram_tensor("ev_h", [NH, K], DT1, kind="ExternalInput").ap()
    evT_h = nc.dram_tensor("evT_h", [K, NH], DT3, kind="ExternalInput").ap()
    evfar = nc.dram_tensor("evfar", [M, K], F32R, kind="ExternalInput").ap()
    evTfar = nc.dram_tensor("evTfar", [K, M], F32R, kind="ExternalInput").ap()
    pk1 = nc.dram_tensor("pk1", [P, PK1_W], F32, kind="ExternalInput").ap()
    pkr = nc.dram_tensor("pkr", [P, PKR_W], F32R, kind="ExternalInput").ap()
    pkw = nc.dram_tensor("pkw", [P, PKW_W], F32R, kind="ExternalInput").ap()
    pkwo = nc.dram_tensor("pkwo", [D, HL * C], F32R, kind="ExternalInput").ap()
    rows = nc.dram_tensor("rows", [6, C], F32, kind="ExternalInput").ap()
    konst = nc.dram_tensor("konst", [2, 512], F32R, kind="ExternalInput").ap()
    out_ap = nc.dram_tensor("out", [NH, C], dt, kind="ExternalOutput").ap()

    RG = [[0, 1], [2, 3], [4, 5], [6, 7]]

    with tile.TileContext(nc) as tc:
        with tc.tile_pool(name="const", bufs=1) as cst, \
             tc.tile_pool(name="mid", bufs=3) as mid, \
             tc.tile_pool(name="p3e", bufs=P3E_BUFS) as p3e, \
             tc.tile_pool(name="dram", bufs=1, space="DRAM") as dram:
            for rep in range(reps):
                # ---- packed params: 4 DMAs instead of ~30 ----
                pk1_t = cst.tile([P, PK1_W], dt, tag="pk1")
                nc.sync.dma_start(pk1_t[:], pk1[:])
                pkr_t = cst.tile([P, PKR_W], F32R, tag="pkr")
                nc.sync.dma_start(pkr_t[:], pkr[:])
                pkw_t = cst.tile([P, PKW_W], F32R, tag="pkw")
                nc.sync.dma_start(pkw_t[:], pkw[:])
                pkwo_t = cst.tile([D, HL * C], F32R, tag="pkwo")
                nc.sync.dma_start(pkwo_t[:], pkwo[:])

                def p1(name):
                    o, w = _PK1[name]
                    return pk1_t[:, o:o + w]
                massT_t, mfarT_t = p1("massT"), p1("mfarT")
                maskq_t, bkm_t = p1("maskq"), p1("bkm")
                gnw_t, gnb_t = p1("gnw"), p1("gnb")
                evals_t, bq_t = p1("evals"), p1("bq")
                gsum_t = pkr_t[:, 0:16]
                gbp_t = pkr_t[:, 16:16 + P]
                wq_t = [pkw_t[:, j * P:(j + 1) * P] for j in range(2)]
                wk_t = [pkw_t[:, 2 * P + j * P:2 * P + (j + 1) * P] for j in range(2)]
                wv_t = [pkw_t[:, 4 * P + j * C:4 * P + (j + 1) * C] for j in range(2)]
                wo_t = [pkwo_t[0:D, h * C:(h + 1) * C] for h in range(HL)]

                ones512 = cst.tile([P, 512], F32R, tag="ones512")
                nc.sync.dma_start(ones512[:], konst[0:1, :].to_broadcast([P, 512]))
                # row params broadcast over partitions (DMA does the broadcast)
                tin_b = cst.tile([P, C], dt, tag="tinb")
                nc.sync.dma_start(tin_b[:], rows[0:1, :].to_broadcast([P, C]))
                nc.vector.tensor_scalar_max(tin_b[:], tin_b[:], 1e-8)
                tout_b = cst.tile([P, C], dt, tag="toutb")
                nc.sync.dma_start(tout_b[:], rows[1:2, :].to_broadcast([P, C]))
                nc.vector.tensor_scalar_max(tout_b[:], tout_b[:], 1e-8)
                outw_b = cst.tile([P, C], dt, tag="outwb")
                nc.sync.dma_start(outw_b[:], rows[2:3, :].to_broadcast([P, C]))
                nc.vector.tensor_scalar_max(outw_b[:], outw_b[:], 1e-8)
                bv_b = cst.tile([P, P], dt, tag="bvb")
                nc.sync.dma_start(bv_b[:], rows[3:4, 0:P].to_broadcast([P, P]))
                bo_b = cst.tile([P, C], dt, tag="bob")
                nc.sync.dma_start(bo_b[:], rows[4:5, :].to_broadcast([P, C]))

                # coefs = exp(-evals x t)
                coef_in = cst.tile([P, C], dt, tag="coefin")
                nc.vector.tensor_tensor(coef_in[:], evals_t.to_broadcast([P, C]),
                                        tin_b[:], MULT)
                nc.scalar.activation(coef_in[:], coef_in[:], AF.Exp, scale=-1.0)
                coef_out = cst.tile([P, C], dt, tag="coefout")
                nc.vector.tensor_tensor(coef_out[:], evals_t.to_broadcast([P, C]),
                                        tout_b[:], MULT)
                nc.scalar.activation(coef_out[:], coef_out[:], AF.Exp, scale=-1.0)

                W33 = HL * (D + 1)
                if 2 in phases:
                    # vaug: one tile, ones-filled; v blocks written later
                    vaug = cst.tile([P, NMK * W33], F32R, tag="vaug")
                    for mc in range(NMK):
                        nc.sync.dma_start(vaug[:, mc * W33:(mc + 1) * W33],
                                          konst[0:1, 0:W33].to_broadcast([P, W33]))

                # =============== PHASE 1: to_basis (N-split) ===============
                with tc.tile_pool(name="p1x", bufs=3) as p1x, \
                     tc.tile_pool(name="p1e", bufs=3) as p1e, \
                     tc.tile_pool(name="ps1", bufs=1, space="PSUM") as ps1:
                    if 2 in phases:
                        # evfar: one load; Gram + column sums for spectral GN;
                        # then mass-scale in place (zspec use)
                        gram_ps = ps1.tile([K, K], dt, tag="gram")
                        s_ps = ps1.tile([K, 2], dt, tag="sps")
                        ef_all = cst.tile([P, NMK, K], F32R, tag="efall")
                        nc.sync.dma_start(
                            ef_all[:], evfar[:, :].rearrange("(m p) k -> p m k", p=P))
                        for mc in range(NMK):
                            nc.tensor.matmul(gram_ps[:], ef_all[:, mc, :], ef_all[:, mc, :],
                                             start=(mc == 0), stop=(mc == NMK - 1))
                            nc.tensor.matmul(s_ps[:], ef_all[:, mc, :], ones512[:, 0:2],
                                             start=(mc == 0), stop=(mc == NMK - 1))
                        for mc in range(NMK):
                            nc.vector.tensor_scalar_mul(ef_all[:, mc, :], ef_all[:, mc, :],
                                                        mfarT_t[:, mc:mc + 1])
                        gram_sb = cst.tile([K, K], F32R, tag="gram_sb")
                        nc.scalar.copy(gram_sb[:], gram_ps[:])
                        s_sb = cst.tile([K, 2], F32R, tag="s_sb")
                        nc.vector.tensor_copy(out=s_sb[:], in_=s_ps[:])
                        evTfar_t = cst.tile([K, M], F32R, tag="evTfar")
                        nc.sync.dma_start(evTfar_t[:], evTfar[:])

                    xspec_ps = ps1.tile([K, C], dt, tag="xspec")
                    ng = NCH // P1G
                    for g in range(ng):
                        # per-partition contiguous row blocks: [p, j] = row p*P1G+j
                        # (evecs load issues first: the mass-scale chains off it)
                        et = p1e.tile([P, P1G, K], DT1, tag="e8")
                        nc.sync.dma_start(
                            et[:], ev_h[g * P1G * P:(g + 1) * P1G * P, :]
                            .rearrange("(p j) k -> p j k", j=P1G))
                        xt = p1x.tile([P, P1G, C], DT1, tag="x8")
                        nc.sync.dma_start(
                            xt[:], x_h[g * P1G * P:(g + 1) * P1G * P, :]
                            .rearrange("(p j) c -> p j c", j=P1G))
                        nc.vector.tensor_tensor(
                            et[:], et[:],
                            massT_t[:, g * P1G:(g + 1) * P1G, None]
                            .to_broadcast([P, P1G, K]), MULT)
                        for j in range(P1G):
                            nc.tensor.matmul(xspec_ps[:], et[:, j, :], xt[:, j, :],
                                             start=(g == 0 and j == 0),
                                             stop=(g == ng - 1 and j == P1G - 1))
                    xspec_sb = cst.tile([K, C], dt, tag="xspec_sb")
                    nc.scalar.copy(xspec_sb[:], xspec_ps[:])

                # AllReduce #1 (pair): xspec; evT prefetch issues right after so
                # the transfers fill the middle phase's otherwise-idle DMA
                ar1_in = dram.tile([K, C], dt, tag="ar1in")
                ar1_out = dram.tile([K, C], dt, tag="ar1out")
                nc.sync.dma_start(ar1_in[:], xspec_sb[:])
                if single or noar:
                    nc.sync.dma_start(ar1_out[:], ar1_in[:])
                else:
                    nc.gpsimd.collective_compute(
                        "AllReduce", ADD, replica_groups=RG,
                        ins=[ar1_in[:].opt()], outs=[ar1_out[:].opt()])
                xspec_sum = cst.tile([K, C], dt, tag="xspec_sum")
                nc.sync.dma_start(xspec_sum[:], ar1_out[:])
                if 3 in phases:
                    p3et = [p3e.tile([K, P3G * P], DT3, tag="evt8", bufs=P3E_BUFS,
                                     name=f"p3et{g}") for g in range(NCH // P3G)]
                    for g in range(P3E_BUFS):
                        nc.sync.dma_start(p3et[g][:],
                                          evT_h[:, g * P3G * P:(g + 1) * P3G * P])

                spec1 = cst.tile([K, C], F32R, tag="spec1")
                nc.vector.tensor_tensor(spec1[:], coef_in[:], xspec_sum[:], MULT)

                if 2 in phases:
                    # =============== MIDDLE ===============
                    with tc.tile_pool(name="psm", bufs=1, space="PSUM") as psm:
                        # ---- spectral GN stats ----
                        t1 = psm.tile([K, C], dt, tag="mm256", bufs=2)
                        nc.tensor.matmul(t1[:], gram_sb[:], spec1[:],
                                         start=True, stop=True)
                        sq = cst.tile([K, C], F32R, tag="sq")
                        nc.vector.tensor_tensor(sq[:], spec1[:], t1[:], MULT)
                        # fp32r matmuls need even free dims: stats come out as
                        # duplicated column pairs, compacted below
                        stat_ps = psm.tile([P, 8], dt, tag="mm256", bufs=2)
                        for cc in range(2):
                            nc.tensor.matmul(stat_ps[:, 2 * cc:2 * cc + 2],
                                             spec1[:, cc * P:(cc + 1) * P], s_sb[:],
                                             start=True, stop=True)
                            nc.tensor.matmul(stat_ps[:, 4 + 2 * cc:6 + 2 * cc],
                                             sq[:, cc * P:(cc + 1) * P], ones512[:, 0:2],
                                             start=True, stop=True)
                        stat_mq = cst.tile([P, 8], F32R, tag="statmq")
                        nc.vector.tensor_copy(out=stat_mq[:], in_=stat_ps[:])
                        pg = psm.tile([16, 8], dt, tag="mm256", bufs=2)
                        nc.tensor.matmul(pg[:], gsum_t, stat_mq[:], start=True, stop=True)
                        inv = 1.0 / (M * 8)
                        mu = cst.tile([16, 2], dt, tag="mu")
                        nc.vector.tensor_scalar_mul(mu[:, 0:1], pg[:, 0:1], inv)
                        nc.vector.tensor_scalar_mul(mu[:, 1:2], pg[:, 2:3], inv)
                        ms = cst.tile([16, 2], dt, tag="ms")
                        nc.vector.tensor_scalar_mul(ms[:, 0:1], pg[:, 4:5], inv)
                        nc.vector.tensor_scalar_mul(ms[:, 1:2], pg[:, 6:7], inv)
                        var = cst.tile([16, 2], dt, tag="var")
                        nc.vector.tensor_tensor(var[:], mu[:], mu[:], MULT)
                        nc.vector.tensor_sub(var[:], ms[:], var[:])
                        nc.vector.tensor_scalar_add(var[:], var[:], EPS)
                        std = cst.tile([16, 2], dt, tag="std")
                        nc.scalar.activation(std[:], var[:], AF.Sqrt)
                        rstd = cst.tile([16, 2], dt, tag="rstd")
                        nc.vector.reciprocal(rstd[:], std[:])
                        stats_sb = cst.tile([P, 4], F32R, tag="stats")
                        nc.sync.dma_start(stats_sb[:], konst[1:2, 0:4].to_broadcast([P, 4]))
                        nc.vector.tensor_copy(out=stats_sb[0:16, 0:2], in_=mu[:])
                        nc.vector.tensor_copy(out=stats_sb[0:16, 2:4], in_=rstd[:])
                        pbc = psm.tile([P, 4], dt, tag="mm256", bufs=2)
                        nc.tensor.matmul(pbc[:], gbp_t, stats_sb[:], start=True, stop=True)
                        A = cst.tile([P, 2], dt, tag="gnA")
                        nc.vector.tensor_tensor(A[:], pbc[:, 2:4], gnw_t, MULT)
                        Bt = cst.tile([P, 2], dt, tag="gnB")
                        nc.vector.tensor_tensor(Bt[:], pbc[:, 0:2], A[:], MULT)
                        nc.vector.tensor_sub(Bt[:], gnb_t, Bt[:])

                        # ---- x_farT [C(2 chunks of 128), M], then GN affine ----
                        xfT = [cst.tile([P, M], F32R, tag=f"xfT{cc}", name=f"xfT{cc}")
                               for cc in range(2)]
                        for cc in range(2):
                            for mh in range(2):
                                px = psm.tile([P, 512], dt, tag="psc2", bufs=2)
                                nc.tensor.matmul(px[:], spec1[:, cc * P:(cc + 1) * P],
                                                 evTfar_t[:, mh * 512:(mh + 1) * 512],
                                                 start=True, stop=True)
                                if mh == 0:
                                    nc.scalar.copy(xfT[cc][:, mh * 512:(mh + 1) * 512], px[:])
                                else:
                                    nc.vector.tensor_copy(
                                        out=xfT[cc][:, mh * 512:(mh + 1) * 512], in_=px[:])
                        for cc in range(2):
                            nc.vector.scalar_tensor_tensor(
                                xfT[cc][:], xfT[cc][:], A[:, cc:cc + 1],
                                Bt[:, cc:cc + 1].to_broadcast([P, M]), MULT, ADD)

                        # ---- qT / kTpad projections ----
                        qT = cst.tile([P, M], F32R, tag="qT")
                        kTpad = [cst.tile([P, M], F32R, tag=f"kTpad{h}",
                                          name=f"kTpad{h}") for h in range(HL)]
                        for mh in range(2):
                            pq = psm.tile([P, 512], dt, tag="psc2", bufs=2)
                            pk = psm.tile([P, 512], dt, tag="psc2", bufs=2)
                            for cin in range(2):
                                nc.tensor.matmul(pq[:], wq_t[cin],
                                                 xfT[cin][:, mh * 512:(mh + 1) * 512],
                                                 start=(cin == 0), stop=(cin == 1))
                            for cin in range(2):
                                nc.tensor.matmul(pk[:], wk_t[cin],
                                                 xfT[cin][:, mh * 512:(mh + 1) * 512],
                                                 start=(cin == 0), stop=(cin == 1))
                            nc.vector.tensor_tensor(qT[:, mh * 512:(mh + 1) * 512], pq[:],
                                                    bq_t.to_broadcast([P, 512]), ADD)
                            for h in range(HL):
                                # masked write: rows of head h get k+bk, others 0
                                nc.scalar.activation(
                                    kTpad[h][:, mh * 512:(mh + 1) * 512], pk[:, :],
                                    AF.Identity, bias=bkm_t[:, h:h + 1],
                                    scale=maskq_t[:, h:h + 1])

                        # ---- v (natural, my-half cols first) into vaug blocks ----
                        for mc in range(NMK):
                            pv = psm.tile([P, C], dt, tag="mm256", bufs=2)
                            for cin in range(2):
                                nc.tensor.matmul(pv[:], xfT[cin][:, mc * P:(mc + 1) * P],
                                                 wv_t[cin],
                                                 start=(cin == 0), stop=(cin == 1))
                            for h in range(HL):
                                nc.vector.tensor_add(
                                    vaug[:, mc * W33 + h * (D + 1):
                                            mc * W33 + h * (D + 1) + D],
                                    pv[:, h * D:(h + 1) * D],
                                    bv_b[:, h * D:(h + 1) * D])

                        # ---- attention: scoresT -> 1024-wide exp -> PV ----
                        # per-head tiles at partition base 0 (psum matmul writes
                        # must start at 0/32/64 and stay in-bounds)
                        oTh = [cst.tile([D, M], F32R, tag=f"oTh{h}",
                                        name=f"oTh{h}") for h in range(HL)]
                        den_sb = cst.tile([D + 1, HL * M], dt, tag="densb")
                        den_d = [dram.tile([1, M], dt, tag=f"dend{h}",
                                           name=f"dend{h}") for h in range(HL)]
                        scl = 1.0 / np.sqrt(D)
                        for h in range(HL):
                            po = [psm.tile([D + 1, 512], dt, tag="po", bufs=2,
                                           name=f"po{h}_{q}") for q in range(2)]
                            pts = [None] * NMK
                            for mkc in range(NMK):
                                psc = psm.tile([P, 1024], dt, tag="psc2", bufs=2)
                                for q in range(2):
                                    nc.tensor.matmul(
                                        psc[:, q * 512:(q + 1) * 512],
                                        kTpad[h][:, mkc * P:(mkc + 1) * P],
                                        qT[:, q * 512:(q + 1) * 512],
                                        start=True, stop=True)
                                pt = mid.tile([P, 1024], F32R, tag="ptile", bufs=4,
                                              name=f"pt{mkc}")
                                # two 512-wide exps: one ACT read must stay
                                # within a single PSUM bank
                                for q in range(2):
                                    nc.scalar.activation(pt[:, q * 512:(q + 1) * 512],
                                                         psc[:, q * 512:(q + 1) * 512],
                                                         AF.Exp, scale=scl)
                                pts[mkc] = pt
                                if mkc > 0:
                                    for q in range(2):
                                        nc.tensor.matmul(
                                            po[q][:],
                                            vaug[:, (mkc - 1) * W33 + h * (D + 1):
                                                    (mkc - 1) * W33 + (h + 1) * (D + 1)],
                                            pts[mkc - 1][:, q * 512:(q + 1) * 512],
                                            start=(mkc - 1 == 0), stop=False)
                            for q in range(2):
                                nc.tensor.matmul(
                                    po[q][:],
                                    vaug[:, (NMK - 1) * W33 + h * (D + 1):
                                            (NMK - 1) * W33 + (h + 1) * (D + 1)],
                                    pts[NMK - 1][:, q * 512:(q + 1) * 512],
                                    start=False, stop=True)
                            for q in range(2):
                                sl = slice(q * 512, (q + 1) * 512)
                                nc.vector.tensor_copy(out=oTh[h][:, sl],
                                                      in_=po[q][0:D, :])
                                nc.vector.tensor_copy(
                                    out=den_sb[D:D + 1, h * M + q * 512:
                                               h * M + (q + 1) * 512],
                                    in_=po[q][D:D + 1, :])
                            # denom bounce + normalize (overlaps next head)
                            nc.sync.dma_start(den_d[h][:],
                                              den_sb[D:D + 1, h * M:(h + 1) * M])
                            db = mid.tile([D, M], dt, tag="denb", bufs=2,
                                          name=f"denb{h}")
                            nc.sync.dma_start(db[:],
                                              den_d[h][0:1, :].to_broadcast([D, M]))
                            nc.vector.reciprocal(db[:], db[:])
                            nc.vector.tensor_tensor(oTh[h][:], oTh[h][:], db[:], MULT)

                        # ---- out-projection partial + zspec partial ----
                        zspec_ps = psm.tile([K, C], dt, tag="po", bufs=2)
                        for mc in range(NMK):
                            pa = psm.tile([P, C], dt, tag="mm256", bufs=2)
                            for h in range(HL):
                                nc.tensor.matmul(pa[:], oTh[h][:, mc * P:(mc + 1) * P],
                                                 wo_t[h],
                                                 start=(h == 0), stop=(h == HL - 1))
                            at = mid.tile([P, C], F32R, tag="atile")
                            nc.vector.tensor_add(at[:], pa[:], bo_b[:])
                            nc.tensor.matmul(zspec_ps[:], ef_all[:, mc, :], at[:],
                                             start=(mc == 0), stop=(mc == NMK - 1))
                        zspec_sb = cst.tile([K, C], dt, tag="zspec_sb")
                        nc.scalar.copy(zspec_sb[:], zspec_ps[:])

                    # AllReduce #2 (pair): zspec
                    ar2_in = dram.tile([K, C], dt, tag="ar2in")
                    ar2_out = dram.tile([K, C], dt, tag="ar2out")
                    nc.sync.dma_start(ar2_in[:], zspec_sb[:])
                    if single or noar:
                        nc.sync.dma_start(ar2_out[:], ar2_in[:])
                    else:
                        nc.gpsimd.collective_compute(
                            "AllReduce", ADD, replica_groups=RG,
                            ins=[ar2_in[:].opt()], outs=[ar2_out[:].opt()])
                    zspec_sum = cst.tile([K, C], dt, tag="zspec_sum")
                    nc.sync.dma_start(zspec_sum[:], ar2_out[:])

                    spec2 = cst.tile([K, C], DT3, tag="spec2")
                    nc.vector.tensor_tensor(spec2[:], coef_out[:], zspec_sum[:], MULT)
                    nc.vector.tensor_tensor(spec2[:], spec2[:], outw_b[:], MULT)

                if 3 not in phases:
                    nc.sync.dma_start(out_ap[0:P, :], xspec_sum[:])
                if 3 in phases:
                    # =============== PHASE 3: from_basis (N-split) ===============
                    with tc.tile_pool(name="p3o", bufs=3) as p3o, \
                         tc.tile_pool(name="ps3", bufs=6, space="PSUM") as ps3:
                        ng = NCH // P3G
                        for g in range(ng):
                            if g >= P3E_BUFS:
                                nc.sync.dma_start(
                                    p3et[g][:],
                                    evT_h[:, g * P3G * P:(g + 1) * P3G * P])
                            et = p3et[g]
                            ot = p3o.tile([P, P3G, C], dt, tag="out8")
                            for j in range(P3G):
                                pp = ps3.tile([P, C], dt, tag="p3")
                                nc.tensor.matmul(pp[:], et[:, j * P:(j + 1) * P],
                                                 spec2[:], start=True, stop=True)
                                if j % 2 == 0:
                                    nc.vector.tensor_copy(out=ot[:, j, :], in_=pp[:])
                                else:
                                    nc.scalar.copy(ot[:, j, :], pp[:])
                            nc.sync.dma_start(
                                out_ap[g * P3G * P:(g + 1) * P3G * P, :]
                                .rearrange("(p j) c -> p j c", j=P3G),
                                ot[:])

    nc.compile()
    return nc


_PROG = None


def _get_prog():
    global _PROG
    if _PROG is None:
        _PROG = _build()
    return _PROG


def make_in_maps(x, mass, evals, evecs, far_idx, diff_in_t, diff_out_t, gn_w, gn_b,
                 Wq, bq, Wk, bk, Wv, bv, Wo, bo, out_w):
    """Host-side (layout-only) prep of the 8 per-core input dicts."""
    import ml_dtypes
    f32 = np.float32
    np1 = ml_dtypes.bfloat16 if DT1 == BF16 else f32
    np3 = ml_dtypes.bfloat16 if DT3 == BF16 else f32
    asf = lambda a: np.ascontiguousarray(a, dtype=f32)
    x = np.asarray(x, dtype=f32)
    mass = np.asarray(mass, dtype=f32)
    evals = np.asarray(evals, dtype=f32)
    evecs = np.asarray(evecs, dtype=f32)
    far_idx = np.asarray(far_idx)
    gsum_m = np.zeros((P, 16), f32)
    gsum_m[np.arange(P), np.arange(P) // 8] = 1.0
    gbp_m = np.zeros((P, P), f32)
    gbp_m[np.arange(P) // 8, np.arange(P)] = 1.0
    maskq_m = (np.arange(P)[:, None] // D == np.arange(HL)[None, :]).astype(f32)
    konst_m = np.stack([np.ones(512, f32), np.zeros(512, f32)])
    in_maps = []
    for core in range(8):
        b, half = core // 2, core % 2
        rs = slice(half * NH, (half + 1) * NH)
        hc = slice(half * P, (half + 1) * P)        # my C_ATT columns / heads
        oc = slice((1 - half) * P, (2 - half) * P)  # partner's columns
        fi = far_idx[b]
        ev_far = evecs[b][fi]                       # [M, K]
        pk1_m = np.zeros((P, PK1_W), f32)
        pk1_m[:, 0:NCH] = (mass[b, rs].reshape(NCH // P1G, P, P1G)
                           .transpose(1, 0, 2).reshape(P, NCH))
        pk1_m[:, 128:128 + NMK] = mass[b][fi].reshape(NMK, P).T
        pk1_m[:, 136:136 + HL] = maskq_m
        pk1_m[:, 140:140 + HL] = maskq_m * np.asarray(bk)[hc][:, None]
        pk1_m[:, 144:146] = np.asarray(gn_w).reshape(2, P).T
        pk1_m[:, 146:148] = np.asarray(gn_b).reshape(2, P).T
        pk1_m[:, 148] = evals[b]
        pk1_m[:, 149] = np.asarray(bq)[hc]
        pkr_m = np.concatenate([gsum_m, gbp_m], axis=1)
        pkw_m = np.concatenate(
            [np.asarray(Wq)[:, hc].reshape(2, P, P).transpose(1, 0, 2).reshape(P, 2 * P),
             np.asarray(Wk)[:, hc].reshape(2, P, P).transpose(1, 0, 2).reshape(P, 2 * P),
             np.concatenate([np.asarray(Wv)[:, hc], np.asarray(Wv)[:, oc]], axis=1)
             .reshape(2, P, C).transpose(1, 0, 2).reshape(P, 2 * C)], axis=1)
        pkwo_m = np.asarray(Wo)[hc].reshape(HL, D, C).transpose(1, 0, 2) \
            .reshape(D, HL * C)
        rows_m = np.zeros((6, C), f32)
        rows_m[0] = np.asarray(diff_in_t)
        rows_m[1] = np.asarray(diff_out_t)
        rows_m[2] = np.asarray(out_w)
        rows_m[3, 0:P] = np.asarray(bv)[hc]
        rows_m[4] = 0.5 * np.asarray(bo)
        m = {
            "x_h": np.ascontiguousarray(x[b, rs], dtype=np1),
            "ev_h": np.ascontiguousarray(evecs[b, rs], dtype=np1),
            "evT_h": np.ascontiguousarray(
                evecs[b, rs].T.reshape(K, NCH // P3G, P, P3G)
                .transpose(0, 1, 3, 2).reshape(K, NH), dtype=np3),
            "evfar": asf(ev_far),
            "evTfar": asf(ev_far.T),
            "pk1": pk1_m,
            "pkr": pkr_m,
            "pkw": asf(pkw_m),
            "pkwo": asf(pkwo_m),
            "rows": rows_m,
            "konst": konst_m,
        }
        in_maps.append(m)
    return in_maps


def kernel(**inputs):
    nc = _get_prog()
    in_maps = make_in_maps(**inputs)
    res = run_bass_kernel_spmd(nc, in_maps, core_ids=list(range(8)))
    out = np.empty((B, N, C), np.float32)
    for core in range(8):
        b, half = core // 2, core % 2
        out[b, half * NH:(half + 1) * NH] = res.results[core]["out"]
    return out


# revision 26
# speedup vs baseline: 29507.1099x; 1.6547x over previous
"""DiffusedFarthestAttention Trainium2 kernel (8-core SPMD Bass/Tile).

Decomposition (B=4 batches x 2 halves -> 8 cores; pair (2b, 2b+1) handles batch b):
  Phase 1: to_basis, N-split.  xspec_partial[K,C] = sum_n (evecs[n,:]*mass[n])^T x[n,:]
           over this core's 16384 rows; AllReduce over the pair.  The evfar Gram
           matrix + column sums (for spectral GroupNorm stats) hide under P1's DMA.
  Middle (head-split, 4 heads per core; all 8 programs identical, split lives in
           the DATA): spectral coefs, GroupNorm stats computed spectrally from
           spec1 and the Gram matrix, x_farT via spec1-matmuls, q/k/v projections
           in transposed layouts, per-head scoresT -> single 1024-wide exp ->
           ones-augmented PV giving softmax denominators, denominator broadcast
           via small DRAM bounce, out-projection partial (bo/2 each) ->
           zspec_partial; AllReduce #2.
  Phase 3: from_basis, N-split.  out rows = evT_chunk^T @ (coefs_out*zspec*out_w).
           evT fully prefetched during the middle; per-partition-contiguous
           row-block layouts keep every DMA at >=2KB descriptors.

Heavy matmuls run as bfloat16 (P1/P3 streams) or float32r (FP22) elsewhere.
Host-side prep is layout-only (transposes, gathers by far_idx, reshapes, dtype
casts); all arithmetic happens on device.
"""

import numpy as np

import concourse.bass as bass
import concourse.mybir as mybir
import concourse.tile as tile
from concourse import bacc
from concourse.bass_utils import run_bass_kernel_spmd

B, N, K, M = 4, 32768, 128, 1024
C = 256          # C_IN = C_OUT = C_ATT
H, D = 8, 32     # heads, head dim
EPS = 1e-6
P = 128
NH = N // 2      # rows per core
NCH = NH // P    # 128 n-chunks per core
P1G = 8          # n-chunks per P1 group
P3G = 8          # n-chunks per P3 group
HL = H // 2      # heads per core
NMK = M // P
F32 = mybir.dt.float32
F32R = mybir.dt.float32r
BF16 = mybir.dt.bfloat16
DT1 = BF16       # phase-1 stream dtype (x, evecs natural)
DT3 = BF16       # phase-3 stream dtype (evT, spec2w)
P3E_BUFS = 16 if DT3 == BF16 else 10
ADD = mybir.AluOpType.add
MULT = mybir.AluOpType.mult
AF = mybir.ActivationFunctionType

# packed f32 param columns (pk1)
_PK1 = dict(massT=(0, NCH), mfarT=(128, NMK), maskq=(136, HL), bkm=(140, HL),
            gnw=(144, 2), gnb=(146, 2), evals=(148, 1), bq=(149, 1))
PK1_W = 150
# packed f32r matrix columns (pkr)
_PKR = dict(gsum=(0, 16), gbp=(16, P))
PKR_W = 144
# packed f32r weight columns (pkw): wq0|wq1|wk0|wk1|wv0|wv1
PKW_W = 2 * P + 2 * P + 2 * C


def _build(single=False, phases=(1, 2, 3), reps=1, noar=False):
    """single=True: 1-core variant with AllReduce -> local copy, for TimelineSim."""
    nc = bacc.Bacc("TRN2", target_bir_lowering=False, debug=False,
                   enable_asserts=False, num_devices=1 if single else 8)
    dt = F32
    x_h = nc.dram_tensor("x_h", [NH, C], DT1, kind="ExternalInput").ap()
    ev_h = nc.dram_tensor("ev_h", [NH, K], DT1, kind="ExternalInput").ap()
    evT_h = nc.dram_tensor("evT_h", [K, NH], DT3, kind="ExternalInput").ap()
    evfar = nc.dram_tensor("evfar", [M, K], F32R, kind="ExternalInput").ap()
    evTfar = nc.dram_tensor("evTfar", [K, M], F32R, kind="ExternalInput").ap()
    pk1 = nc.dram_tensor("pk1", [P, PK1_W], F32, kind="ExternalInput").ap()
    pkr = nc.dram_tensor("pkr", [P, PKR_W], F32R, kind="ExternalInput").ap()
    pkw = nc.dram_tensor("pkw", [P, PKW_W], F32R, kind="ExternalInput").ap()
    pkwo = nc.dram_tensor("pkwo", [D, HL * C], F32R, kind="ExternalInput").ap()
    rows = nc.dram_tensor("rows", [6, C], F32, kind="ExternalInput").ap()
    konst = nc.dram_tensor("konst", [2, 512], F32R, kind="ExternalInput").ap()
    out_ap = nc.dram_tensor("out", [NH, C], dt, kind="ExternalOutput").ap()

    RG = [[0, 1], [2, 3], [4, 5], [6, 7]]

    with tile.TileContext(nc) as tc:
        with tc.tile_pool(name="const", bufs=1) as cst, \
             tc.tile_pool(name="mid", bufs=3) as mid, \
             tc.tile_pool(name="p3e", bufs=P3E_BUFS) as p3e, \
             tc.tile_pool(name="dram", bufs=1, space="DRAM") as dram:
            for rep in range(reps):
                # ---- packed params: 4 DMAs instead of ~30 ----
                pk1_t = cst.tile([P, PK1_W], dt, tag="pk1")
                nc.sync.dma_start(pk1_t[:], pk1[:])
                pkr_t = cst.tile([P, PKR_W], F32R, tag="pkr")
                nc.sync.dma_start(pkr_t[:], pkr[:])
                pkw_t = cst.tile([P, PKW_W], F32R, tag="pkw")
                nc.sync.dma_start(pkw_t[:], pkw[:])
                pkwo_t = cst.tile([D, HL * C], F32R, tag="pkwo")
                nc.sync.dma_start(pkwo_t[:], pkwo[:])

                def p1(name):
                    o, w = _PK1[name]
                    return pk1_t[:, o:o + w]
                massT_t, mfarT_t = p1("massT"), p1("mfarT")
                maskq_t, bkm_t = p1("maskq"), p1("bkm")
                gnw_t, gnb_t = p1("gnw"), p1("gnb")
                evals_t, bq_t = p1("evals"), p1("bq")
                gsum_t = pkr_t[:, 0:16]
                gbp_t = pkr_t[:, 16:16 + P]
                wq_t = [pkw_t[:, j * P:(j + 1) * P] for j in range(2)]
                wk_t = [pkw_t[:, 2 * P + j * P:2 * P + (j + 1) * P] for j in range(2)]
                wv_t = [pkw_t[:, 4 * P + j * C:4 * P + (j + 1) * C] for j in range(2)]
                wo_t = [pkwo_t[0:D, h * C:(h + 1) * C] for h in range(HL)]

                ones512 = cst.tile([P, 512], F32R, tag="ones512")
                nc.sync.dma_start(ones512[:], konst[0:1, :].to_broadcast([P, 512]))
                # row params broadcast over partitions (DMA does the broadcast)
                tin_b = cst.tile([P, C], dt, tag="tinb")
                nc.sync.dma_start(tin_b[:], rows[0:1, :].to_broadcast([P, C]))
                nc.vector.tensor_scalar_max(tin_b[:], tin_b[:], 1e-8)
                tout_b = cst.tile([P, C], dt, tag="toutb")
                nc.sync.dma_start(tout_b[:], rows[1:2, :].to_broadcast([P, C]))
                nc.vector.tensor_scalar_max(tout_b[:], tout_b[:], 1e-8)
                outw_b = cst.tile([P, C], dt, tag="outwb")
                nc.sync.dma_start(outw_b[:], rows[2:3, :].to_broadcast([P, C]))
                nc.vector.tensor_scalar_max(outw_b[:], outw_b[:], 1e-8)
                bv_b = cst.tile([P, P], dt, tag="bvb")
                nc.sync.dma_start(bv_b[:], rows[3:4, 0:P].to_broadcast([P, P]))
                bo_b = cst.tile([P, C], dt, tag="bob")
                nc.sync.dma_start(bo_b[:], rows[4:5, :].to_broadcast([P, C]))

                # coefs = exp(-evals x t)
                coef_in = cst.tile([P, C], dt, tag="coefin")
                nc.vector.tensor_tensor(coef_in[:], evals_t.to_broadcast([P, C]),
                                        tin_b[:], MULT)
                nc.scalar.activation(coef_in[:], coef_in[:], AF.Exp, scale=-1.0)
                coef_out = cst.tile([P, C], dt, tag="coefout")
                nc.vector.tensor_tensor(coef_out[:], evals_t.to_broadcast([P, C]),
                                        tout_b[:], MULT)
                nc.scalar.activation(coef_out[:], coef_out[:], AF.Exp, scale=-1.0)

                W33 = HL * (D + 1)
                if 2 in phases:
                    # vaug: one tile, ones-filled; v blocks written later
                    vaug = cst.tile([P, NMK * W33], F32R, tag="vaug")
                    for mc in range(NMK):
                        nc.sync.dma_start(vaug[:, mc * W33:(mc + 1) * W33],
                                          konst[0:1, 0:W33].to_broadcast([P, W33]))

                # =============== PHASE 1: to_basis (N-split) ===============
                with tc.tile_pool(name="p1x", bufs=3) as p1x, \
                     tc.tile_pool(name="p1e", bufs=3) as p1e, \
                     tc.tile_pool(name="ps1", bufs=1, space="PSUM") as ps1:
                    if 2 in phases:
                        # evfar: one load; Gram + column sums for spectral GN;
                        # then mass-scale in place (zspec use)
                        gram_ps = ps1.tile([K, K], dt, tag="gram")
                        s_ps = ps1.tile([K, 2], dt, tag="sps")
                        ef_all = cst.tile([P, NMK, K], F32R, tag="efall")
                        nc.sync.dma_start(
                            ef_all[:], evfar[:, :].rearrange("(m p) k -> p m k", p=P))
                        for mc in range(NMK):
                            nc.tensor.matmul(gram_ps[:], ef_all[:, mc, :], ef_all[:, mc, :],
                                             start=(mc == 0), stop=(mc == NMK - 1))
                            nc.tensor.matmul(s_ps[:], ef_all[:, mc, :], ones512[:, 0:2],
                                             start=(mc == 0), stop=(mc == NMK - 1))
                        for mc in range(NMK):
                            nc.vector.tensor_scalar_mul(ef_all[:, mc, :], ef_all[:, mc, :],
                                                        mfarT_t[:, mc:mc + 1])
                        gram_sb = cst.tile([K, K], F32R, tag="gram_sb")
                        nc.scalar.copy(gram_sb[:], gram_ps[:])
                        s_sb = cst.tile([K, 2], F32R, tag="s_sb")
                        nc.vector.tensor_copy(out=s_sb[:], in_=s_ps[:])
                        evTfar_t = cst.tile([K, M], F32R, tag="evTfar")
                        nc.sync.dma_start(evTfar_t[:], evTfar[:])

                    xspec_ps = ps1.tile([K, C], dt, tag="xspec")
                    ng = NCH // P1G
                    for g in range(ng):
                        # per-partition contiguous row blocks: [p, j] = row p*P1G+j
                        # (evecs load issues first: the mass-scale chains off it)
                        et = p1e.tile([P, P1G, K], DT1, tag="e8")
                        nc.sync.dma_start(
                            et[:], ev_h[g * P1G * P:(g + 1) * P1G * P, :]
                            .rearrange("(p j) k -> p j k", j=P1G))
                        xt = p1x.tile([P, P1G, C], DT1, tag="x8")
                        nc.sync.dma_start(
                            xt[:], x_h[g * P1G * P:(g + 1) * P1G * P, :]
                            .rearrange("(p j) c -> p j c", j=P1G))
                        nc.vector.tensor_tensor(
                            et[:], et[:],
                            massT_t[:, g * P1G:(g + 1) * P1G, None]
                            .to_broadcast([P, P1G, K]), MULT)
                        for j in range(P1G):
                            nc.tensor.matmul(xspec_ps[:], et[:, j, :], xt[:, j, :],
                                             start=(g == 0 and j == 0),
                                             stop=(g == ng - 1 and j == P1G - 1))
                    xspec_sb = cst.tile([K, C], dt, tag="xspec_sb")
                    nc.scalar.copy(xspec_sb[:], xspec_ps[:])

                # AllReduce #1 (pair): xspec; evT prefetch issues right after so
                # the transfers fill the middle phase's otherwise-idle DMA
                ar1_in = dram.tile([K, C], dt, tag="ar1in")
                ar1_out = dram.tile([K, C], dt, tag="ar1out")
                nc.sync.dma_start(ar1_in[:], xspec_sb[:])
                if single or noar:
                    nc.sync.dma_start(ar1_out[:], ar1_in[:])
                else:
                    nc.gpsimd.collective_compute(
                        "AllReduce", ADD, replica_groups=RG,
                        ins=[ar1_in[:].opt()], outs=[ar1_out[:].opt()])
                xspec_sum = cst.tile([K, C], dt, tag="xspec_sum")
                nc.sync.dma_start(xspec_sum[:], ar1_out[:])
                if 3 in phases:
                    p3et = [p3e.tile([K, P3G * P], DT3, tag="evt8", bufs=P3E_BUFS,
                                     name=f"p3et{g}") for g in range(NCH // P3G)]
                    for g in range(P3E_BUFS):
                        nc.sync.dma_start(p3et[g][:],
                                          evT_h[:, g * P3G * P:(g + 1) * P3G * P])

                spec1 = cst.tile([K, C], F32R, tag="spec1")
                nc.vector.tensor_tensor(spec1[:], coef_in[:], xspec_sum[:], MULT)

                if 2 in phases:
                    # =============== MIDDLE ===============
                    with tc.tile_pool(name="psm", bufs=1, space="PSUM") as psm:
                        # ---- spectral GN stats ----
                        t1 = psm.tile([K, C], dt, tag="mm256", bufs=2)
                        nc.tensor.matmul(t1[:], gram_sb[:], spec1[:],
                                         start=True, stop=True)
                        sq = cst.tile([K, C], F32R, tag="sq")
                        nc.vector.tensor_tensor(sq[:], spec1[:], t1[:], MULT)
                        # fp32r matmuls need even free dims: stats come out as
                        # duplicated column pairs, compacted below
                        stat_ps = psm.tile([P, 8], dt, tag="mm256", bufs=2)
                        for cc in range(2):
                            nc.tensor.matmul(stat_ps[:, 2 * cc:2 * cc + 2],
                                             spec1[:, cc * P:(cc + 1) * P], s_sb[:],
                                             start=True, stop=True)
                            nc.tensor.matmul(stat_ps[:, 4 + 2 * cc:6 + 2 * cc],
                                             sq[:, cc * P:(cc + 1) * P], ones512[:, 0:2],
                                             start=True, stop=True)
                        stat_mq = cst.tile([P, 8], F32R, tag="statmq")
                        nc.vector.tensor_copy(out=stat_mq[:], in_=stat_ps[:])
                        pg = psm.tile([16, 8], dt, tag="mm256", bufs=2)
                        nc.tensor.matmul(pg[:], gsum_t, stat_mq[:], start=True, stop=True)
                        inv = 1.0 / (M * 8)
                        mu = cst.tile([16, 2], dt, tag="mu")
                        nc.vector.tensor_scalar_mul(mu[:, 0:1], pg[:, 0:1], inv)
                        nc.vector.tensor_scalar_mul(mu[:, 1:2], pg[:, 2:3], inv)
                        ms = cst.tile([16, 2], dt, tag="ms")
                        nc.vector.tensor_scalar_mul(ms[:, 0:1], pg[:, 4:5], inv)
                        nc.vector.tensor_scalar_mul(ms[:, 1:2], pg[:, 6:7], inv)
                        var = cst.tile([16, 2], dt, tag="var")
                        nc.vector.tensor_tensor(var[:], mu[:], mu[:], MULT)
                        nc.vector.tensor_sub(var[:], ms[:], var[:])
                        nc.vector.tensor_scalar_add(var[:], var[:], EPS)
                        std = cst.tile([16, 2], dt, tag="std")
                        nc.scalar.activation(std[:], var[:], AF.Sqrt)
                        rstd = cst.tile([16, 2], dt, tag="rstd")
                        nc.vector.reciprocal(rstd[:], std[:])
                        stats_sb = cst.tile([P, 4], F32R, tag="stats")
                        nc.sync.dma_start(stats_sb[:], konst[1:2, 0:4].to_broadcast([P, 4]))
                        nc.vector.tensor_copy(out=stats_sb[0:16, 0:2], in_=mu[:])
                        nc.vector.tensor_copy(out=stats_sb[0:16, 2:4], in_=rstd[:])
                        pbc = psm.tile([P, 4], dt, tag="mm256", bufs=2)
                        nc.tensor.matmul(pbc[:], gbp_t, stats_sb[:], start=True, stop=True)
                        A = cst.tile([P, 2], dt, tag="gnA")
                        nc.vector.tensor_tensor(A[:], pbc[:, 2:4], gnw_t, MULT)
                        Bt = cst.tile([P, 2], dt, tag="gnB")
                        nc.vector.tensor_tensor(Bt[:], pbc[:, 0:2], A[:], MULT)
                        nc.vector.tensor_sub(Bt[:], gnb_t, Bt[:])

                        # ---- x_farT [C(2 chunks of 128), M], then GN affine ----
                        xfT = [cst.tile([P, M], F32R, tag=f"xfT{cc}", name=f"xfT{cc}")
                               for cc in range(2)]
                        for cc in range(2):
                            for mh in range(2):
                                px = psm.tile([P, 512], dt, tag="psc2", bufs=2)
                                nc.tensor.matmul(px[:], spec1[:, cc * P:(cc + 1) * P],
                                                 evTfar_t[:, mh * 512:(mh + 1) * 512],
                                                 start=True, stop=True)
                                nc.vector.tensor_copy(
                                    out=xfT[cc][:, mh * 512:(mh + 1) * 512], in_=px[:])
                        for cc in range(2):
                            nc.vector.scalar_tensor_tensor(
                                xfT[cc][:], xfT[cc][:], A[:, cc:cc + 1],
                                Bt[:, cc:cc + 1].to_broadcast([P, M]), MULT, ADD)

                        # ---- qT / kTpad projections ----
                        qT = cst.tile([P, M], F32R, tag="qT")
                        kTpad = [cst.tile([P, M], F32R, tag=f"kTpad{h}",
                                          name=f"kTpad{h}") for h in range(HL)]
                        for mh in range(2):
                            pq = psm.tile([P, 512], dt, tag="psc2", bufs=2)
                            pk = psm.tile([P, 512], dt, tag="psc2", bufs=2)
                            for cin in range(2):
                                nc.tensor.matmul(pq[:], wq_t[cin],
                                                 xfT[cin][:, mh * 512:(mh + 1) * 512],
                                                 start=(cin == 0), stop=(cin == 1))
                            for cin in range(2):
                                nc.tensor.matmul(pk[:], wk_t[cin],
                                                 xfT[cin][:, mh * 512:(mh + 1) * 512],
                                                 start=(cin == 0), stop=(cin == 1))
                            nc.vector.tensor_tensor(qT[:, mh * 512:(mh + 1) * 512], pq[:],
                                                    bq_t.to_broadcast([P, 512]), ADD)
                            for h in range(HL):
                                # masked write: rows of head h get k+bk, others 0
                                # (DVE keeps the ACT free for the exp burst)
                                nc.vector.scalar_tensor_tensor(
                                    kTpad[h][:, mh * 512:(mh + 1) * 512], pk[:, :],
                                    maskq_t[:, h:h + 1],
                                    bkm_t[:, h:h + 1].to_broadcast([P, 512]), MULT, ADD)

                        # ---- v (natural, my-half cols first) into vaug blocks ----
                        for mc in range(NMK):
                            pv = psm.tile([P, C], dt, tag="mm256", bufs=2)
                            for cin in range(2):
                                nc.tensor.matmul(pv[:], xfT[cin][:, mc * P:(mc + 1) * P],
                                                 wv_t[cin],
                                                 start=(cin == 0), stop=(cin == 1))
                            for h in range(HL):
                                nc.vector.tensor_add(
                                    vaug[:, mc * W33 + h * (D + 1):
                                            mc * W33 + h * (D + 1) + D],
                                    pv[:, h * D:(h + 1) * D],
                                    bv_b[:, h * D:(h + 1) * D])

                        # ---- attention: scoresT -> 1024-wide exp -> PV ----
                        # per-head tiles at partition base 0 (psum matmul writes
                        # must start at 0/32/64 and stay in-bounds)
                        oTh = [cst.tile([D, M], F32R, tag=f"oTh{h}",
                                        name=f"oTh{h}") for h in range(HL)]
                        den_sb = cst.tile([D + 1, HL * M], dt, tag="densb")
                        den_d = [dram.tile([1, M], dt, tag=f"dend{h}",
                                           name=f"dend{h}") for h in range(HL)]
                        scl = 1.0 / np.sqrt(D)
                        for h in range(HL):
                            po = [psm.tile([D + 1, 512], dt, tag="po", bufs=2,
                                           name=f"po{h}_{q}") for q in range(2)]
                            pts = [None] * NMK
                            for mkc in range(NMK):
                                psc = psm.tile([P, 1024], dt, tag="psc2", bufs=2)
                                for q in range(2):
                                    nc.tensor.matmul(
                                        psc[:, q * 512:(q + 1) * 512],
                                        kTpad[h][:, mkc * P:(mkc + 1) * P],
                                        qT[:, q * 512:(q + 1) * 512],
                                        start=True, stop=True)
                                pt = mid.tile([P, 1024], F32R, tag="ptile", bufs=4,
                                              name=f"pt{mkc}")
                                # two 512-wide exps: one ACT read must stay
                                # within a single PSUM bank
                                for q in range(2):
                                    nc.scalar.activation(pt[:, q * 512:(q + 1) * 512],
                                                         psc[:, q * 512:(q + 1) * 512],
                                                         AF.Exp, scale=scl)
                                pts[mkc] = pt
                                if mkc > 0:
                                    for q in range(2):
                                        nc.tensor.matmul(
                                            po[q][:],
                                            vaug[:, (mkc - 1) * W33 + h * (D + 1):
                                                    (mkc - 1) * W33 + (h + 1) * (D + 1)],
                                            pts[mkc - 1][:, q * 512:(q + 1) * 512],
                                            start=(mkc - 1 == 0), stop=False)
                            for q in range(2):
                                nc.tensor.matmul(
                                    po[q][:],
                                    vaug[:, (NMK - 1) * W33 + h * (D + 1):
                                            (NMK - 1) * W33 + (h + 1) * (D + 1)],
                                    pts[NMK - 1][:, q * 512:(q + 1) * 512],
                                    start=False, stop=True)
                            for q in range(2):
                                sl = slice(q * 512, (q + 1) * 512)
                                nc.vector.tensor_copy(out=oTh[h][:, sl],
                                                      in_=po[q][0:D, :])
                                nc.vector.tensor_copy(
                                    out=den_sb[D:D + 1, h * M + q * 512:
                                               h * M + (q + 1) * 512],
                                    in_=po[q][D:D + 1, :])
                            # denom bounce + normalize (overlaps next head)
                            nc.sync.dma_start(den_d[h][:],
                                              den_sb[D:D + 1, h * M:(h + 1) * M])
                            db = mid.tile([D, M], dt, tag="denb", bufs=2,
                                          name=f"denb{h}")
                            nc.sync.dma_start(db[:],
                                              den_d[h][0:1, :].to_broadcast([D, M]))
                            nc.vector.reciprocal(db[:], db[:])
                            nc.vector.tensor_tensor(oTh[h][:], oTh[h][:], db[:], MULT)

                        # ---- out-projection partial + zspec partial ----
                        zspec_ps = psm.tile([K, C], dt, tag="po", bufs=2)
                        for mc in range(NMK):
                            pa = psm.tile([P, C], dt, tag="mm256", bufs=2)
                            for h in range(HL):
                                nc.tensor.matmul(pa[:], oTh[h][:, mc * P:(mc + 1) * P],
                                                 wo_t[h],
                                                 start=(h == 0), stop=(h == HL - 1))
                            at = mid.tile([P, C], F32R, tag="atile")
                            nc.vector.tensor_add(at[:], pa[:], bo_b[:])
                            nc.tensor.matmul(zspec_ps[:], ef_all[:, mc, :], at[:],
                                             start=(mc == 0), stop=(mc == NMK - 1))
                        zspec_sb = cst.tile([K, C], dt, tag="zspec_sb")
                        nc.scalar.copy(zspec_sb[:], zspec_ps[:])

                    # AllReduce #2 (pair): zspec
                    ar2_in = dram.tile([K, C], dt, tag="ar2in")
                    ar2_out = dram.tile([K, C], dt, tag="ar2out")
                    nc.sync.dma_start(ar2_in[:], zspec_sb[:])
                    if single or noar:
                        nc.sync.dma_start(ar2_out[:], ar2_in[:])
                    else:
                        nc.gpsimd.collective_compute(
                            "AllReduce", ADD, replica_groups=RG,
                            ins=[ar2_in[:].opt()], outs=[ar2_out[:].opt()])
                    zspec_sum = cst.tile([K, C], dt, tag="zspec_sum")
                    nc.sync.dma_start(zspec_sum[:], ar2_out[:])

                    spec2 = cst.tile([K, C], DT3, tag="spec2")
                    nc.vector.tensor_tensor(spec2[:], coef_out[:], zspec_sum[:], MULT)
                    nc.vector.tensor_tensor(spec2[:], spec2[:], outw_b[:], MULT)

                if 3 not in phases:
                    nc.sync.dma_start(out_ap[0:P, :], xspec_sum[:])
                if 3 in phases:
                    # =============== PHASE 3: from_basis (N-split) ===============
                    with tc.tile_pool(name="p3o", bufs=3) as p3o, \
                         tc.tile_pool(name="ps3", bufs=6, space="PSUM") as ps3:
                        ng = NCH // P3G
                        for g in range(ng):
                            if g >= P3E_BUFS:
                                nc.sync.dma_start(
                                    p3et[g][:],
                                    evT_h[:, g * P3G * P:(g + 1) * P3G * P])
                            et = p3et[g]
                            ot = p3o.tile([P, P3G, C], dt, tag="out8")
                            for j in range(P3G):
                                pp = ps3.tile([P, C], dt, tag="p3")
                                nc.tensor.matmul(pp[:], et[:, j * P:(j + 1) * P],
                                                 spec2[:], start=True, stop=True)
                                if j % 2 == 0:
                                    nc.vector.tensor_copy(out=ot[:, j, :], in_=pp[:])
                                else:
                                    nc.scalar.copy(ot[:, j, :], pp[:])
                            nc.sync.dma_start(
                                out_ap[g * P3G * P:(g + 1) * P3G * P, :]
                                .rearrange("(p j) c -> p j c", j=P3G),
                                ot[:])

    nc.compile()
    return nc


_PROG = None


def _get_prog():
    global _PROG
    if _PROG is None:
        _PROG = _build()
    return _PROG


def make_in_maps(x, mass, evals, evecs, far_idx, diff_in_t, diff_out_t, gn_w, gn_b,
                 Wq, bq, Wk, bk, Wv, bv, Wo, bo, out_w):
    """Host-side (layout-only) prep of the 8 per-core input dicts."""
    import ml_dtypes
    f32 = np.float32
    np1 = ml_dtypes.bfloat16 if DT1 == BF16 else f32
    np3 = ml_dtypes.bfloat16 if DT3 == BF16 else f32
    asf = lambda a: np.ascontiguousarray(a, dtype=f32)
    x = np.asarray(x, dtype=f32)
    mass = np.asarray(mass, dtype=f32)
    evals = np.asarray(evals, dtype=f32)
    evecs = np.asarray(evecs, dtype=f32)
    far_idx = np.asarray(far_idx)
    gsum_m = np.zeros((P, 16), f32)
    gsum_m[np.arange(P), np.arange(P) // 8] = 1.0
    gbp_m = np.zeros((P, P), f32)
    gbp_m[np.arange(P) // 8, np.arange(P)] = 1.0
    maskq_m = (np.arange(P)[:, None] // D == np.arange(HL)[None, :]).astype(f32)
    konst_m = np.stack([np.ones(512, f32), np.zeros(512, f32)])
    in_maps = []
    for core in range(8):
        b, half = core // 2, core % 2
        rs = slice(half * NH, (half + 1) * NH)
        hc = slice(half * P, (half + 1) * P)        # my C_ATT columns / heads
        oc = slice((1 - half) * P, (2 - half) * P)  # partner's columns
        fi = far_idx[b]
        ev_far = evecs[b][fi]                       # [M, K]
        pk1_m = np.zeros((P, PK1_W), f32)
        pk1_m[:, 0:NCH] = (mass[b, rs].reshape(NCH // P1G, P, P1G)
                           .transpose(1, 0, 2).reshape(P, NCH))
        pk1_m[:, 128:128 + NMK] = mass[b][fi].reshape(NMK, P).T
        pk1_m[:, 136:136 + HL] = maskq_m
        pk1_m[:, 140:140 + HL] = maskq_m * np.asarray(bk)[hc][:, None]
        pk1_m[:, 144:146] = np.asarray(gn_w).reshape(2, P).T
        pk1_m[:, 146:148] = np.asarray(gn_b).reshape(2, P).T
        pk1_m[:, 148] = evals[b]
        pk1_m[:, 149] = np.asarray(bq)[hc]
        pkr_m = np.concatenate([gsum_m, gbp_m], axis=1)
        pkw_m = np.concatenate(
            [np.asarray(Wq)[:, hc].reshape(2, P, P).transpose(1, 0, 2).reshape(P, 2 * P),
             np.asarray(Wk)[:, hc].reshape(2, P, P).transpose(1, 0, 2).reshape(P, 2 * P),
             np.concatenate([np.asarray(Wv)[:, hc], np.asarray(Wv)[:, oc]], axis=1)
             .reshape(2, P, C).transpose(1, 0, 2).reshape(P, 2 * C)], axis=1)
        pkwo_m = np.asarray(Wo)[hc].reshape(HL, D, C).transpose(1, 0, 2) \
            .reshape(D, HL * C)
        rows_m = np.zeros((6, C), f32)
        rows_m[0] = np.asarray(diff_in_t)
        rows_m[1] = np.asarray(diff_out_t)
        rows_m[2] = np.asarray(out_w)
        rows_m[3, 0:P] = np.asarray(bv)[hc]
        rows_m[4] = 0.5 * np.asarray(bo)
        m = {
            "x_h": np.ascontiguousarray(x[b, rs], dtype=np1),
            "ev_h": np.ascontiguousarray(evecs[b, rs], dtype=np1),
            "evT_h": np.ascontiguousarray(
                evecs[b, rs].T.reshape(K, NCH // P3G, P, P3G)
                .transpose(0, 1, 3, 2).reshape(K, NH), dtype=np3),
            "evfar": asf(ev_far),
            "evTfar": asf(ev_far.T),
            "pk1": pk1_m,
            "pkr": pkr_m,
            "pkw": asf(pkw_m),
            "pkwo": asf(pkwo_m),
            "rows": rows_m,
            "konst": konst_m,
        }
        in_maps.append(m)
    return in_maps


def kernel(**inputs):
    nc = _get_prog()
    in_maps = make_in_maps(**inputs)
    res = run_bass_kernel_spmd(nc, in_maps, core_ids=list(range(8)))
    out = np.empty((B, N, C), np.float32)
    for core in range(8):
        b, half = core // 2, core % 2
        out[b, half * NH:(half + 1) * NH] = res.results[core]["out"]
    return out


# revision 27
# speedup vs baseline: 30676.0413x; 1.0396x over previous
"""DiffusedFarthestAttention Trainium2 kernel (8-core SPMD Bass/Tile).

Decomposition (B=4 batches x 2 halves -> 8 cores; pair (2b, 2b+1) handles batch b):
  Phase 1: to_basis, N-split.  xspec_partial[K,C] = sum_n (evecs[n,:]*mass[n])^T x[n,:]
           over this core's 16384 rows; AllReduce over the pair.  The evfar Gram
           matrix + column sums (for spectral GroupNorm stats) hide under P1's DMA.
  Middle (head-split, 4 heads per core; all 8 programs identical, split lives in
           the DATA): spectral coefs, GroupNorm stats computed spectrally from
           spec1 and the Gram matrix, x_farT via spec1-matmuls, q/k/v projections
           in transposed layouts, per-head scoresT -> single 1024-wide exp ->
           ones-augmented PV giving softmax denominators, denominator broadcast
           via small DRAM bounce, out-projection partial (bo/2 each) ->
           zspec_partial; AllReduce #2.
  Phase 3: from_basis, N-split.  out rows = evT_chunk^T @ (coefs_out*zspec*out_w).
           evT fully prefetched during the middle; per-partition-contiguous
           row-block layouts keep every DMA at >=2KB descriptors.

Heavy matmuls run as bfloat16 (P1/P3 streams) or float32r (FP22) elsewhere.
Host-side prep is layout-only (transposes, gathers by far_idx, reshapes, dtype
casts); all arithmetic happens on device.
"""

import numpy as np

import concourse.bass as bass
import concourse.mybir as mybir
import concourse.tile as tile
from concourse import bacc
from concourse.bass_utils import run_bass_kernel_spmd

B, N, K, M = 4, 32768, 128, 1024
C = 256          # C_IN = C_OUT = C_ATT
H, D = 8, 32     # heads, head dim
EPS = 1e-6
P = 128
NH = N // 2      # rows per core
NCH = NH // P    # 128 n-chunks per core
P1G = 8          # n-chunks per P1 group
P3G = 8          # n-chunks per P3 group
HL = H // 2      # heads per core
NMK = M // P
F32 = mybir.dt.float32
F32R = mybir.dt.float32r
BF16 = mybir.dt.bfloat16
DT1 = BF16       # phase-1 stream dtype (x, evecs natural)
DT3 = BF16       # phase-3 stream dtype (evT, spec2w)
P3E_BUFS = 16 if DT3 == BF16 else 10
ADD = mybir.AluOpType.add
MULT = mybir.AluOpType.mult
AF = mybir.ActivationFunctionType

# packed f32 param columns (pk1)
_PK1 = dict(massT=(0, NCH), mfarT=(128, NMK), maskq=(136, HL), bkm=(140, HL),
            gnw=(144, 2), gnb=(146, 2), evals=(148, 1), bq=(149, 1))
PK1_W = 150
# packed f32r matrix columns (pkr)
_PKR = dict(gsum=(0, 16), gbp=(16, P))
PKR_W = 144
# packed f32r weight columns (pkw): wq0|wq1|wk0|wk1|wv0|wv1
PKW_W = 2 * P + 2 * P + 2 * C


def _build(single=False, phases=(1, 2, 3), reps=1, noar=False):
    """single=True: 1-core variant with AllReduce -> local copy, for TimelineSim."""
    nc = bacc.Bacc("TRN2", target_bir_lowering=False, debug=False,
                   enable_asserts=False, num_devices=1 if single else 8)
    dt = F32
    x_h = nc.dram_tensor("x_h", [NH, C], DT1, kind="ExternalInput").ap()
    ev_h = nc.dram_tensor("ev_h", [NH, K], DT1, kind="ExternalInput").ap()
    evT_h = nc.dram_tensor("evT_h", [K, NH], DT3, kind="ExternalInput").ap()
    evfar = nc.dram_tensor("evfar", [M, K], F32R, kind="ExternalInput").ap()
    evTfar = nc.dram_tensor("evTfar", [K, M], F32R, kind="ExternalInput").ap()
    pk1 = nc.dram_tensor("pk1", [P, PK1_W], F32, kind="ExternalInput").ap()
    pkr = nc.dram_tensor("pkr", [P, PKR_W], F32R, kind="ExternalInput").ap()
    pkw = nc.dram_tensor("pkw", [P, PKW_W], F32R, kind="ExternalInput").ap()
    pkwo = nc.dram_tensor("pkwo", [D, HL * C], F32R, kind="ExternalInput").ap()
    rows = nc.dram_tensor("rows", [6, C], F32, kind="ExternalInput").ap()
    konst = nc.dram_tensor("konst", [2, 512], F32R, kind="ExternalInput").ap()
    out_ap = nc.dram_tensor("out", [NH, C], dt, kind="ExternalOutput").ap()

    RG = [[0, 1], [2, 3], [4, 5], [6, 7]]

    with tile.TileContext(nc) as tc:
        with tc.tile_pool(name="const", bufs=1) as cst, \
             tc.tile_pool(name="mid", bufs=3) as mid, \
             tc.tile_pool(name="p3e", bufs=P3E_BUFS) as p3e, \
             tc.tile_pool(name="dram", bufs=1, space="DRAM") as dram:
            for rep in range(reps):
                # ---- packed params: 4 DMAs instead of ~30 ----
                pk1_t = cst.tile([P, PK1_W], dt, tag="pk1")
                nc.sync.dma_start(pk1_t[:], pk1[:])
                pkr_t = cst.tile([P, PKR_W], F32R, tag="pkr")
                nc.sync.dma_start(pkr_t[:], pkr[:])
                pkw_t = cst.tile([P, PKW_W], F32R, tag="pkw")
                nc.sync.dma_start(pkw_t[:], pkw[:])
                pkwo_t = cst.tile([D, HL * C], F32R, tag="pkwo")
                nc.sync.dma_start(pkwo_t[:], pkwo[:])

                def p1(name):
                    o, w = _PK1[name]
                    return pk1_t[:, o:o + w]
                massT_t, mfarT_t = p1("massT"), p1("mfarT")
                maskq_t, bkm_t = p1("maskq"), p1("bkm")
                gnw_t, gnb_t = p1("gnw"), p1("gnb")
                evals_t, bq_t = p1("evals"), p1("bq")
                gsum_t = pkr_t[:, 0:16]
                gbp_t = pkr_t[:, 16:16 + P]
                wq_t = [pkw_t[:, j * P:(j + 1) * P] for j in range(2)]
                wk_t = [pkw_t[:, 2 * P + j * P:2 * P + (j + 1) * P] for j in range(2)]
                wv_t = [pkw_t[:, 4 * P + j * C:4 * P + (j + 1) * C] for j in range(2)]
                wo_t = [pkwo_t[0:D, h * C:(h + 1) * C] for h in range(HL)]

                ones512 = cst.tile([P, 512], F32R, tag="ones512")
                nc.sync.dma_start(ones512[:], konst[0:1, :].to_broadcast([P, 512]))
                # row params broadcast over partitions (DMA does the broadcast)
                tin_b = cst.tile([P, C], dt, tag="tinb")
                nc.sync.dma_start(tin_b[:], rows[0:1, :].to_broadcast([P, C]))
                nc.vector.tensor_scalar_max(tin_b[:], tin_b[:], 1e-8)
                tout_b = cst.tile([P, C], dt, tag="toutb")
                nc.sync.dma_start(tout_b[:], rows[1:2, :].to_broadcast([P, C]))
                nc.vector.tensor_scalar_max(tout_b[:], tout_b[:], 1e-8)
                outw_b = cst.tile([P, C], dt, tag="outwb")
                nc.sync.dma_start(outw_b[:], rows[2:3, :].to_broadcast([P, C]))
                nc.vector.tensor_scalar_max(outw_b[:], outw_b[:], 1e-8)
                bv_b = cst.tile([P, P], dt, tag="bvb")
                nc.sync.dma_start(bv_b[:], rows[3:4, 0:P].to_broadcast([P, P]))
                bo_b = cst.tile([P, C], dt, tag="bob")
                nc.sync.dma_start(bo_b[:], rows[4:5, :].to_broadcast([P, C]))

                # coefs = exp(-evals x t)
                coef_in = cst.tile([P, C], dt, tag="coefin")
                nc.vector.tensor_tensor(coef_in[:], evals_t.to_broadcast([P, C]),
                                        tin_b[:], MULT)
                nc.scalar.activation(coef_in[:], coef_in[:], AF.Exp, scale=-1.0)
                coef_out = cst.tile([P, C], dt, tag="coefout")
                nc.vector.tensor_tensor(coef_out[:], evals_t.to_broadcast([P, C]),
                                        tout_b[:], MULT)
                nc.scalar.activation(coef_out[:], coef_out[:], AF.Exp, scale=-1.0)

                W33 = HL * (D + 1)
                if 2 in phases:
                    # vaug: one tile, ones-filled; v blocks written later
                    vaug = cst.tile([P, NMK * W33], F32R, tag="vaug")
                    for mc in range(NMK):
                        nc.sync.dma_start(vaug[:, mc * W33:(mc + 1) * W33],
                                          konst[0:1, 0:W33].to_broadcast([P, W33]))

                # =============== PHASE 1: to_basis (N-split) ===============
                with tc.tile_pool(name="p1x", bufs=3) as p1x, \
                     tc.tile_pool(name="p1e", bufs=3) as p1e, \
                     tc.tile_pool(name="ps1", bufs=1, space="PSUM") as ps1:
                    if 2 in phases:
                        # evfar: one load; Gram + column sums for spectral GN;
                        # then mass-scale in place (zspec use)
                        gram_ps = ps1.tile([K, K], dt, tag="gram")
                        s_ps = ps1.tile([K, 2], dt, tag="sps")
                        ef_all = cst.tile([P, NMK, K], F32R, tag="efall")
                        nc.sync.dma_start(
                            ef_all[:], evfar[:, :].rearrange("(m p) k -> p m k", p=P))
                        for mc in range(NMK):
                            nc.tensor.matmul(gram_ps[:], ef_all[:, mc, :], ef_all[:, mc, :],
                                             start=(mc == 0), stop=(mc == NMK - 1))
                            nc.tensor.matmul(s_ps[:], ef_all[:, mc, :], ones512[:, 0:2],
                                             start=(mc == 0), stop=(mc == NMK - 1))
                        for mc in range(NMK):
                            nc.vector.tensor_scalar_mul(ef_all[:, mc, :], ef_all[:, mc, :],
                                                        mfarT_t[:, mc:mc + 1])
                        gram_sb = cst.tile([K, K], F32R, tag="gram_sb")
                        nc.scalar.copy(gram_sb[:], gram_ps[:])
                        s_sb = cst.tile([K, 2], F32R, tag="s_sb")
                        nc.vector.tensor_copy(out=s_sb[:], in_=s_ps[:])
                        evTfar_t = cst.tile([K, M], F32R, tag="evTfar")
                        nc.sync.dma_start(evTfar_t[:], evTfar[:])

                    xspec_ps = ps1.tile([K, C], dt, tag="xspec")
                    ng = NCH // P1G
                    for g in range(ng):
                        # per-partition contiguous row blocks: [p, j] = row p*P1G+j
                        # (evecs load issues first: the mass-scale chains off it)
                        et = p1e.tile([P, P1G, K], DT1, tag="e8")
                        nc.sync.dma_start(
                            et[:], ev_h[g * P1G * P:(g + 1) * P1G * P, :]
                            .rearrange("(p j) k -> p j k", j=P1G))
                        xt = p1x.tile([P, P1G, C], DT1, tag="x8")
                        nc.sync.dma_start(
                            xt[:], x_h[g * P1G * P:(g + 1) * P1G * P, :]
                            .rearrange("(p j) c -> p j c", j=P1G))
                        for j in range(P1G):
                            # 2D per-chunk scale: matmul j waits ~100ns, not the
                            # whole group's 3D scale
                            cix = g * P1G + j
                            nc.vector.tensor_tensor(
                                et[:, j, :], et[:, j, :],
                                massT_t[:, cix:cix + 1].to_broadcast([P, K]), MULT)
                            nc.tensor.matmul(xspec_ps[:], et[:, j, :], xt[:, j, :],
                                             start=(g == 0 and j == 0),
                                             stop=(g == ng - 1 and j == P1G - 1))
                    xspec_sb = cst.tile([K, C], dt, tag="xspec_sb")
                    nc.scalar.copy(xspec_sb[:], xspec_ps[:])

                # AllReduce #1 (pair): xspec; evT prefetch issues right after so
                # the transfers fill the middle phase's otherwise-idle DMA
                ar1_in = dram.tile([K, C], dt, tag="ar1in")
                ar1_out = dram.tile([K, C], dt, tag="ar1out")
                nc.sync.dma_start(ar1_in[:], xspec_sb[:])
                if single or noar:
                    nc.sync.dma_start(ar1_out[:], ar1_in[:])
                else:
                    nc.gpsimd.collective_compute(
                        "AllReduce", ADD, replica_groups=RG,
                        ins=[ar1_in[:].opt()], outs=[ar1_out[:].opt()])
                xspec_sum = cst.tile([K, C], dt, tag="xspec_sum")
                nc.sync.dma_start(xspec_sum[:], ar1_out[:])
                if 3 in phases:
                    p3et = [p3e.tile([K, P3G * P], DT3, tag="evt8", bufs=P3E_BUFS,
                                     name=f"p3et{g}") for g in range(NCH // P3G)]
                    for g in range(P3E_BUFS):
                        nc.sync.dma_start(p3et[g][:],
                                          evT_h[:, g * P3G * P:(g + 1) * P3G * P])

                spec1 = cst.tile([K, C], F32R, tag="spec1")
                nc.vector.tensor_tensor(spec1[:], coef_in[:], xspec_sum[:], MULT)

                if 2 in phases:
                    # =============== MIDDLE ===============
                    with tc.tile_pool(name="psm", bufs=1, space="PSUM") as psm:
                        # ---- spectral GN stats ----
                        t1 = psm.tile([K, C], dt, tag="mm256", bufs=2)
                        nc.tensor.matmul(t1[:], gram_sb[:], spec1[:],
                                         start=True, stop=True)
                        sq = cst.tile([K, C], F32R, tag="sq")
                        nc.vector.tensor_tensor(sq[:], spec1[:], t1[:], MULT)
                        # fp32r matmuls need even free dims: stats come out as
                        # duplicated column pairs, compacted below
                        stat_ps = psm.tile([P, 8], dt, tag="mm256", bufs=2)
                        for cc in range(2):
                            nc.tensor.matmul(stat_ps[:, 2 * cc:2 * cc + 2],
                                             spec1[:, cc * P:(cc + 1) * P], s_sb[:],
                                             start=True, stop=True)
                            nc.tensor.matmul(stat_ps[:, 4 + 2 * cc:6 + 2 * cc],
                                             sq[:, cc * P:(cc + 1) * P], ones512[:, 0:2],
                                             start=True, stop=True)
                        stat_mq = cst.tile([P, 8], F32R, tag="statmq")
                        nc.vector.tensor_copy(out=stat_mq[:], in_=stat_ps[:])
                        pg = psm.tile([16, 8], dt, tag="mm256", bufs=2)
                        nc.tensor.matmul(pg[:], gsum_t, stat_mq[:], start=True, stop=True)
                        inv = 1.0 / (M * 8)
                        mu = cst.tile([16, 2], dt, tag="mu")
                        nc.vector.tensor_scalar_mul(mu[:, 0:1], pg[:, 0:1], inv)
                        nc.vector.tensor_scalar_mul(mu[:, 1:2], pg[:, 2:3], inv)
                        ms = cst.tile([16, 2], dt, tag="ms")
                        nc.vector.tensor_scalar_mul(ms[:, 0:1], pg[:, 4:5], inv)
                        nc.vector.tensor_scalar_mul(ms[:, 1:2], pg[:, 6:7], inv)
                        var = cst.tile([16, 2], dt, tag="var")
                        nc.vector.tensor_tensor(var[:], mu[:], mu[:], MULT)
                        nc.vector.tensor_sub(var[:], ms[:], var[:])
                        nc.vector.tensor_scalar_add(var[:], var[:], EPS)
                        std = cst.tile([16, 2], dt, tag="std")
                        nc.scalar.activation(std[:], var[:], AF.Sqrt)
                        rstd = cst.tile([16, 2], dt, tag="rstd")
                        nc.vector.reciprocal(rstd[:], std[:])
                        stats_sb = cst.tile([P, 4], F32R, tag="stats")
                        nc.sync.dma_start(stats_sb[:], konst[1:2, 0:4].to_broadcast([P, 4]))
                        nc.vector.tensor_copy(out=stats_sb[0:16, 0:2], in_=mu[:])
                        nc.vector.tensor_copy(out=stats_sb[0:16, 2:4], in_=rstd[:])
                        pbc = psm.tile([P, 4], dt, tag="mm256", bufs=2)
                        nc.tensor.matmul(pbc[:], gbp_t, stats_sb[:], start=True, stop=True)
                        A = cst.tile([P, 2], dt, tag="gnA")
                        nc.vector.tensor_tensor(A[:], pbc[:, 2:4], gnw_t, MULT)
                        Bt = cst.tile([P, 2], dt, tag="gnB")
                        nc.vector.tensor_tensor(Bt[:], pbc[:, 0:2], A[:], MULT)
                        nc.vector.tensor_sub(Bt[:], gnb_t, Bt[:])

                        # ---- x_farT [C(2 chunks of 128), M], then GN affine ----
                        xfT = [cst.tile([P, M], F32R, tag=f"xfT{cc}", name=f"xfT{cc}")
                               for cc in range(2)]
                        for cc in range(2):
                            for mh in range(2):
                                px = psm.tile([P, 512], dt, tag="psc2", bufs=2)
                                nc.tensor.matmul(px[:], spec1[:, cc * P:(cc + 1) * P],
                                                 evTfar_t[:, mh * 512:(mh + 1) * 512],
                                                 start=True, stop=True)
                                nc.vector.tensor_copy(
                                    out=xfT[cc][:, mh * 512:(mh + 1) * 512], in_=px[:])
                        for cc in range(2):
                            nc.vector.scalar_tensor_tensor(
                                xfT[cc][:], xfT[cc][:], A[:, cc:cc + 1],
                                Bt[:, cc:cc + 1].to_broadcast([P, M]), MULT, ADD)

                        # ---- qT / kTpad projections ----
                        qT = cst.tile([P, M], F32R, tag="qT")
                        kTpad = [cst.tile([P, M], F32R, tag=f"kTpad{h}",
                                          name=f"kTpad{h}") for h in range(HL)]
                        for mh in range(2):
                            pq = psm.tile([P, 512], dt, tag="psc2", bufs=2)
                            pk = psm.tile([P, 512], dt, tag="psc2", bufs=2)
                            for cin in range(2):
                                nc.tensor.matmul(pq[:], wq_t[cin],
                                                 xfT[cin][:, mh * 512:(mh + 1) * 512],
                                                 start=(cin == 0), stop=(cin == 1))
                            for cin in range(2):
                                nc.tensor.matmul(pk[:], wk_t[cin],
                                                 xfT[cin][:, mh * 512:(mh + 1) * 512],
                                                 start=(cin == 0), stop=(cin == 1))
                            nc.vector.tensor_tensor(qT[:, mh * 512:(mh + 1) * 512], pq[:],
                                                    bq_t.to_broadcast([P, 512]), ADD)
                            for h in range(HL):
                                # masked write: rows of head h get k+bk, others 0
                                # (DVE keeps the ACT free for the exp burst)
                                nc.vector.scalar_tensor_tensor(
                                    kTpad[h][:, mh * 512:(mh + 1) * 512], pk[:, :],
                                    maskq_t[:, h:h + 1],
                                    bkm_t[:, h:h + 1].to_broadcast([P, 512]), MULT, ADD)

                        # ---- v (natural, my-half cols first) into vaug blocks ----
                        for mc in range(NMK):
                            pv = psm.tile([P, C], dt, tag="mm256", bufs=2)
                            for cin in range(2):
                                nc.tensor.matmul(pv[:], xfT[cin][:, mc * P:(mc + 1) * P],
                                                 wv_t[cin],
                                                 start=(cin == 0), stop=(cin == 1))
                            for h in range(HL):
                                nc.vector.tensor_add(
                                    vaug[:, mc * W33 + h * (D + 1):
                                            mc * W33 + h * (D + 1) + D],
                                    pv[:, h * D:(h + 1) * D],
                                    bv_b[:, h * D:(h + 1) * D])

                        # ---- attention: scoresT -> 1024-wide exp -> PV ----
                        # per-head tiles at partition base 0 (psum matmul writes
                        # must start at 0/32/64 and stay in-bounds)
                        oTh = [cst.tile([D, M], F32R, tag=f"oTh{h}",
                                        name=f"oTh{h}") for h in range(HL)]
                        den_sb = cst.tile([D + 1, HL * M], dt, tag="densb")
                        den_d = [dram.tile([1, M], dt, tag=f"dend{h}",
                                           name=f"dend{h}") for h in range(HL)]
                        scl = 1.0 / np.sqrt(D)
                        for h in range(HL):
                            po = [psm.tile([D + 1, 512], dt, tag="po", bufs=2,
                                           name=f"po{h}_{q}") for q in range(2)]
                            pts = [None] * NMK
                            for mkc in range(NMK):
                                psc = psm.tile([P, 1024], dt, tag="psc2", bufs=2)
                                for q in range(2):
                                    nc.tensor.matmul(
                                        psc[:, q * 512:(q + 1) * 512],
                                        kTpad[h][:, mkc * P:(mkc + 1) * P],
                                        qT[:, q * 512:(q + 1) * 512],
                                        start=True, stop=True)
                                pt = mid.tile([P, 1024], F32R, tag="ptile", bufs=4,
                                              name=f"pt{mkc}")
                                # two 512-wide exps: one ACT read must stay
                                # within a single PSUM bank
                                for q in range(2):
                                    nc.scalar.activation(pt[:, q * 512:(q + 1) * 512],
                                                         psc[:, q * 512:(q + 1) * 512],
                                                         AF.Exp, scale=scl)
                                pts[mkc] = pt
                                if mkc > 0:
                                    for q in range(2):
                                        nc.tensor.matmul(
                                            po[q][:],
                                            vaug[:, (mkc - 1) * W33 + h * (D + 1):
                                                    (mkc - 1) * W33 + (h + 1) * (D + 1)],
                                            pts[mkc - 1][:, q * 512:(q + 1) * 512],
                                            start=(mkc - 1 == 0), stop=False)
                            for q in range(2):
                                nc.tensor.matmul(
                                    po[q][:],
                                    vaug[:, (NMK - 1) * W33 + h * (D + 1):
                                            (NMK - 1) * W33 + (h + 1) * (D + 1)],
                                    pts[NMK - 1][:, q * 512:(q + 1) * 512],
                                    start=False, stop=True)
                            for q in range(2):
                                sl = slice(q * 512, (q + 1) * 512)
                                nc.vector.tensor_copy(out=oTh[h][:, sl],
                                                      in_=po[q][0:D, :])
                                nc.vector.tensor_copy(
                                    out=den_sb[D:D + 1, h * M + q * 512:
                                               h * M + (q + 1) * 512],
                                    in_=po[q][D:D + 1, :])
                            # denom bounce + normalize (overlaps next head)
                            nc.sync.dma_start(den_d[h][:],
                                              den_sb[D:D + 1, h * M:(h + 1) * M])
                            db = mid.tile([D, M], dt, tag="denb", bufs=2,
                                          name=f"denb{h}")
                            nc.sync.dma_start(db[:],
                                              den_d[h][0:1, :].to_broadcast([D, M]))
                            nc.vector.reciprocal(db[:], db[:])
                            nc.vector.tensor_tensor(oTh[h][:], oTh[h][:], db[:], MULT)

                        # ---- out-projection partial + zspec partial ----
                        zspec_ps = psm.tile([K, C], dt, tag="po", bufs=2)
                        ats = [None] * NMK
                        for mc in range(NMK):
                            pa = psm.tile([P, C], dt, tag="mm256", bufs=2)
                            for h in range(HL):
                                nc.tensor.matmul(pa[:], oTh[h][:, mc * P:(mc + 1) * P],
                                                 wo_t[h],
                                                 start=(h == 0), stop=(h == HL - 1))
                            at = mid.tile([P, C], F32R, tag="atile", bufs=3,
                                          name=f"at{mc}")
                            nc.vector.tensor_add(at[:], pa[:], bo_b[:])
                            ats[mc] = at
                            if mc > 0:
                                nc.tensor.matmul(zspec_ps[:], ef_all[:, mc - 1, :],
                                                 ats[mc - 1][:],
                                                 start=(mc - 1 == 0), stop=False)
                        nc.tensor.matmul(zspec_ps[:], ef_all[:, NMK - 1, :],
                                         ats[NMK - 1][:], start=False, stop=True)
                        zspec_sb = cst.tile([K, C], dt, tag="zspec_sb")
                        nc.scalar.copy(zspec_sb[:], zspec_ps[:])

                    # AllReduce #2 (pair): zspec
                    ar2_in = dram.tile([K, C], dt, tag="ar2in")
                    ar2_out = dram.tile([K, C], dt, tag="ar2out")
                    nc.sync.dma_start(ar2_in[:], zspec_sb[:])
                    if single or noar:
                        nc.sync.dma_start(ar2_out[:], ar2_in[:])
                    else:
                        nc.gpsimd.collective_compute(
                            "AllReduce", ADD, replica_groups=RG,
                            ins=[ar2_in[:].opt()], outs=[ar2_out[:].opt()])
                    zspec_sum = cst.tile([K, C], dt, tag="zspec_sum")
                    nc.sync.dma_start(zspec_sum[:], ar2_out[:])

                    spec2 = cst.tile([K, C], DT3, tag="spec2")
                    nc.vector.tensor_tensor(spec2[:], coef_out[:], zspec_sum[:], MULT)
                    nc.vector.tensor_tensor(spec2[:], spec2[:], outw_b[:], MULT)

                if 3 not in phases:
                    nc.sync.dma_start(out_ap[0:P, :], xspec_sum[:])
                if 3 in phases:
                    # =============== PHASE 3: from_basis (N-split) ===============
                    with tc.tile_pool(name="p3o", bufs=3) as p3o, \
                         tc.tile_pool(name="ps3", bufs=6, space="PSUM") as ps3:
                        ng = NCH // P3G
                        for g in range(ng):
                            if g >= P3E_BUFS:
                                nc.sync.dma_start(
                                    p3et[g][:],
                                    evT_h[:, g * P3G * P:(g + 1) * P3G * P])
                            et = p3et[g]
                            ot = p3o.tile([P, P3G, C], dt, tag="out8")
                            for j in range(P3G):
                                pp = ps3.tile([P, C], dt, tag="p3")
                                nc.tensor.matmul(pp[:], et[:, j * P:(j + 1) * P],
                                                 spec2[:], start=True, stop=True)
                                if j % 2 == 0:
                                    nc.vector.tensor_copy(out=ot[:, j, :], in_=pp[:])
                                else:
                                    nc.scalar.copy(ot[:, j, :], pp[:])
                            nc.sync.dma_start(
                                out_ap[g * P3G * P:(g + 1) * P3G * P, :]
                                .rearrange("(p j) c -> p j c", j=P3G),
                                ot[:])

    nc.compile()
    return nc


_PROG = None


def _get_prog():
    global _PROG
    if _PROG is None:
        _PROG = _build()
    return _PROG


def make_in_maps(x, mass, evals, evecs, far_idx, diff_in_t, diff_out_t, gn_w, gn_b,
                 Wq, bq, Wk, bk, Wv, bv, Wo, bo, out_w):
    """Host-side (layout-only) prep of the 8 per-core input dicts."""
    import ml_dtypes
    f32 = np.float32
    np1 = ml_dtypes.bfloat16 if DT1 == BF16 else f32
    np3 = ml_dtypes.bfloat16 if DT3 == BF16 else f32
    asf = lambda a: np.ascontiguousarray(a, dtype=f32)
    x = np.asarray(x, dtype=f32)
    mass = np.asarray(mass, dtype=f32)
    evals = np.asarray(evals, dtype=f32)
    evecs = np.asarray(evecs, dtype=f32)
    far_idx = np.asarray(far_idx)
    gsum_m = np.zeros((P, 16), f32)
    gsum_m[np.arange(P), np.arange(P) // 8] = 1.0
    gbp_m = np.zeros((P, P), f32)
    gbp_m[np.arange(P) // 8, np.arange(P)] = 1.0
    maskq_m = (np.arange(P)[:, None] // D == np.arange(HL)[None, :]).astype(f32)
    konst_m = np.stack([np.ones(512, f32), np.zeros(512, f32)])
    in_maps = []
    for core in range(8):
        b, half = core // 2, core % 2
        rs = slice(half * NH, (half + 1) * NH)
        hc = slice(half * P, (half + 1) * P)        # my C_ATT columns / heads
        oc = slice((1 - half) * P, (2 - half) * P)  # partner's columns
        fi = far_idx[b]
        ev_far = evecs[b][fi]                       # [M, K]
        pk1_m = np.zeros((P, PK1_W), f32)
        pk1_m[:, 0:NCH] = (mass[b, rs].reshape(NCH // P1G, P, P1G)
                           .transpose(1, 0, 2).reshape(P, NCH))
        pk1_m[:, 128:128 + NMK] = mass[b][fi].reshape(NMK, P).T
        pk1_m[:, 136:136 + HL] = maskq_m
        pk1_m[:, 140:140 + HL] = maskq_m * np.asarray(bk)[hc][:, None]
        pk1_m[:, 144:146] = np.asarray(gn_w).reshape(2, P).T
        pk1_m[:, 146:148] = np.asarray(gn_b).reshape(2, P).T
        pk1_m[:, 148] = evals[b]
        pk1_m[:, 149] = np.asarray(bq)[hc]
        pkr_m = np.concatenate([gsum_m, gbp_m], axis=1)
        pkw_m = np.concatenate(
            [np.asarray(Wq)[:, hc].reshape(2, P, P).transpose(1, 0, 2).reshape(P, 2 * P),
             np.asarray(Wk)[:, hc].reshape(2, P, P).transpose(1, 0, 2).reshape(P, 2 * P),
             np.concatenate([np.asarray(Wv)[:, hc], np.asarray(Wv)[:, oc]], axis=1)
             .reshape(2, P, C).transpose(1, 0, 2).reshape(P, 2 * C)], axis=1)
        pkwo_m = np.asarray(Wo)[hc].reshape(HL, D, C).transpose(1, 0, 2) \
            .reshape(D, HL * C)
        rows_m = np.zeros((6, C), f32)
        rows_m[0] = np.asarray(diff_in_t)
        rows_m[1] = np.asarray(diff_out_t)
        rows_m[2] = np.asarray(out_w)
        rows_m[3, 0:P] = np.asarray(bv)[hc]
        rows_m[4] = 0.5 * np.asarray(bo)
        m = {
            "x_h": np.ascontiguousarray(x[b, rs], dtype=np1),
            "ev_h": np.ascontiguousarray(evecs[b, rs], dtype=np1),
            "evT_h": np.ascontiguousarray(
                evecs[b, rs].T.reshape(K, NCH // P3G, P, P3G)
                .transpose(0, 1, 3, 2).reshape(K, NH), dtype=np3),
            "evfar": asf(ev_far),
            "evTfar": asf(ev_far.T),
            "pk1": pk1_m,
            "pkr": pkr_m,
            "pkw": asf(pkw_m),
            "pkwo": asf(pkwo_m),
            "rows": rows_m,
            "konst": konst_m,
        }
        in_maps.append(m)
    return in_maps


def kernel(**inputs):
    nc = _get_prog()
    in_maps = make_in_maps(**inputs)
    res = run_bass_kernel_spmd(nc, in_maps, core_ids=list(range(8)))
    out = np.empty((B, N, C), np.float32)
    for core in range(8):
        b, half = core // 2, core % 2
        out[b, half * NH:(half + 1) * NH] = res.results[core]["out"]
    return out
